# revision 1
# baseline (speedup 1.0000x reference)
"""DivergentAttention Trainium2 kernel (8 NeuronCores, Bass/Tile).

Problem: GPT-2 style causal self-attention (B=2, S=2048, D=1024, H=16,
hd=64) where heads 0/1/2 re-weight their attention toward a token region
(first/middle/last third of the sequence) with factor 1.6 and renormalize.

Key identity: softmax(s)*m / sum(softmax(s)*m) == softmax(s + log m), so the
per-head region reweight folds into an additive per-(head, key-position)
bias on the scores -- no second normalization pass needed. Scores are small
(|s|<~5) so the max-subtraction pass is skipped entirely.

Sharding: tensor-parallel over (batch, head-group): core c handles batch
c//4 and heads [4*(c%4), 4*(c%4)+4). Each core computes the QKV projection
for its 4 heads, full causal attention, and its partial c_proj; the host
sums the 8 partials and adds c_proj_b.

Layouts (all transposed so no on-chip transposes are ever needed):
  - hiddenT  [D, S]  (host-transposed)  -> QKV matmuls contract over D;
    the contraction (ko) loop is OUTER with 8 resident PSUM groups so PE
    starts as soon as the first 128-row chunk of hiddenT/w lands.
  - qkT      [4*128, S]: q(h0,h1) | q(h2,h3) | k(h0,h1) | k(h2,h3); head at
    partition offset 64*(h%2) within its 128-tile.
  - scoresT  [sk-tile=128, sq] = kT.T @ qT; causal => only sq >= 128*t is
    computed; the diagonal 128x128 block gets a 0/1 triangular mask
    multiply AFTER the exp (on GPSIMD, all-SBUF, so it never stalls the
    ScalarE exp stream -- exp(-inf)=0 is replaced by exp(s)*0).
  - exp via ScalarE with scale=1/8 and per-partition bias log(mult[h, sk]).
  - v        [S, hd] natural ([128, 16, 4, 65] with a ones column at index
    64) so out.T = v_aug.T @ attnT gives both out.T (rows 0..63) and the
    softmax denominator (row 64) in one accumulation.
  - denominators: copy [65,1024] PSUM->SBUF (frees the accumulation bank
    early), DVE reciprocal, one DRAM bounce + partition-broadcast DMA per
    head (on the gpsimd queue, off the bulk-DMA queue), DVE multiply into
    ao2 [128, 2, S] float32r with heads partition-interleaved (h even ->
    partitions 0..63, h odd -> 64..127).
  - c_proj: two K=128 matmuls per output tile against pw2 [128, 2, D]
    (head-pair rows packed to match ao2).

All matmuls run in float32r (TF32-like, full PE rate at N>=256; measured
~1.5e-4 relative error at K=1024).
"""

import numpy as np

import concourse.bass as bass
import concourse.tile as tile
from concourse import mybir
from concourse import bass_utils, bass2jax

# ---------------------------------------------------------------- constants
B, S, D, H, HD = 2, 2048, 1024, 16, 64
NCORES = 8
HPC = 4              # heads per core
GROUPS = 4           # head groups
FOCUS = 1.6
HEAD_REGION = {0: 0, 1: 1, 2: 2}
DT_R = mybir.dt.float32r
DT_F = mybir.dt.float32

# ------------------------------------------------- walrus multi-wait fixup
# This container's walrus accepts only ONE sync-wait per TPB instruction,
# but Tile attaches one wait per dependency proc. Rewrite the BIR JSON just
# before walrus: hoist all-but-one wait of a multi-wait instruction onto
# standalone same-engine NoOps inserted immediately before it (same-engine
# program order is preserved, so semantics are unchanged).
try:
    import orjson as _json
except ImportError:  # pragma: no cover
    import json as _json

_orig_compile_bir_kernel = bass_utils.compile_bir_kernel
_wfix_counter = [0]


def _fix_bir(bir_json):
    d = _json.loads(bir_json)
    changed = False
    for fn in d.get("functions", []):
        for blk in fn.get("blocks", []):
            out = []
            for inst in blk.get("instructions", []):
                si = inst.get("sync_info")
                if si:
                    waits = si.get("on_wait") or []
                    if len(waits) > 1:
                        changed = True
                        for w in waits[:-1]:
                            _wfix_counter[0] += 1
                            nop = {
                                "engine": inst["engine"],
                                "ins": [],
                                "name": f"I-wfix-{_wfix_counter[0]}",
                                "opcode": "NoOp",
                                "outs": [],
                                "sync_info": {"on_update": [], "on_wait": [w]},
                            }
                            if "debug" in inst:
                                nop["debug"] = inst["debug"]
                            out.append(nop)
                        si["on_wait"] = waits[-1:]
                out.append(inst)
            blk["instructions"] = out
    return _json.dumps(d) if changed else bir_json


def _patched_compile_bir_kernel(bir_json, tmpdir, neff_name="file.neff"):
    return _orig_compile_bir_kernel(_fix_bir(bir_json), tmpdir, neff_name=neff_name)


def _install_waitfix():
    bass_utils.compile_bir_kernel = _patched_compile_bir_kernel
    bass2jax.compile_bir_kernel = _patched_compile_bir_kernel


_install_waitfix()

# ---------------------------------------------------------------- program


def build_program():
    """One SPMD Bass program; per-core differences come in via inputs."""
    nc = bass.Bass()
    NT = S // 128       # 16 sk tiles
    KO = D // 128       # 8 contraction chunks

    hiddenT = nc.dram_tensor("hiddenT", [D, S], DT_R, kind="ExternalInput")
    w_qkv = nc.dram_tensor("w_qkv", [D, 768], DT_R, kind="ExternalInput")
    bqk = nc.dram_tensor("bqk", [128, 4], DT_F, kind="ExternalInput")
    bv_rep = nc.dram_tensor("bv_rep", [128, 256], DT_F, kind="ExternalInput")
    projw = nc.dram_tensor("projw", [128, 2, D], DT_R, kind="ExternalInput")
    diag_mask = nc.dram_tensor("diag_mask", [128, 128], DT_R, kind="ExternalInput")
    logmult = nc.dram_tensor("logmult", [128, HPC, NT], DT_F, kind="ExternalInput")
    out = nc.dram_tensor("out", [S, D], DT_F, kind="ExternalOutput")

    with tile.TileContext(nc) as tc:
        with tc.tile_pool(name="persist", bufs=1) as persist, \
             tc.tile_pool(name="dram", bufs=6, space="DRAM") as dram:

            # ---- persistent SBUF ----
            qk_sb = persist.tile([128, 4, S], DT_R)        # 4 MB
            v_sb = persist.tile([128, NT, HPC, 65], DT_R)  # ~2.1 MB
            ao2 = persist.tile([128, 2, S], DT_R)          # attn_outT, 2 MB
            bqk_sb = persist.tile([128, 4], DT_F)
            bv_sb = persist.tile([128, 256], DT_F)
            pw_sb = persist.tile([128, 2, D], DT_R)        # 1 MB
            dm_sb = persist.tile([128, 128], DT_R)
            lm_sb = persist.tile([128, HPC, NT], DT_F)

            nc.sync.dma_start(bqk_sb, bqk[:, :])
            nc.vector.memset(v_sb[:, :, :, 64:65].bitcast(DT_F), 1.0)

            # ================= phase 1: QKV projection =================
            # ko (contraction) outer, 8 resident PSUM groups: PE consumes
            # each 1.4 MB (hiddenT+w) chunk as it arrives from HBM.
            with tc.tile_pool(name="p1sb", bufs=1) as p1sb, \
                 tc.tile_pool(name="p1ps", bufs=8, space="PSUM") as p1ps:
                hT = p1sb.tile([128, KO, S], DT_R)        # 8 MB
                w_sb = p1sb.tile([128, KO, 768], DT_R)    # 3 MB
                hT_src = hiddenT.rearrange("(ko p) s -> p ko s", p=128)
                w_src = w_qkv.rearrange("(ko p) n -> p ko n", p=128)
                # three-way load split: hiddenT alternates the two HWDGE
                # queues (SP + ACT), w rides the otherwise-idle GPSIMD SWDGE
                # queue, small tensors trail it
                for ko in range(KO):
                    q = nc.sync if ko % 2 == 0 else nc.scalar
                    q.dma_start(hT[:, ko, :], hT_src[:, ko, :])
                    nc.gpsimd.dma_start(w_sb[:, ko, :], w_src[:, ko, :])
                    if ko == 0:
                        nc.sync.dma_start(bv_sb, bv_rep[:, :])
                        nc.gpsimd.dma_start(dm_sb, diag_mask[:, :])
                        nc.gpsimd.dma_start(lm_sb, logmult[:, :, :])
                        nc.gpsimd.dma_start(pw_sb, projw[:, :, :])

                # qT/kT: out[n-tile, s] = w.T @ hiddenT, two rounds of 8 psums
                for rnd in range(2):
                    ps8 = [p1ps.tile([128, 512], DT_F, tag="g", name=f"q{rnd}{i}")
                           for i in range(8)]
                    for ko in range(KO):
                        for i in range(8):
                            nt, sc = (0, 2, 1, 3)[2 * rnd + i // 4], i % 4
                            nc.tensor.matmul(
                                ps8[i],
                                w_sb[:, ko, 128 * nt:128 * nt + 128],
                                hT[:, ko, 512 * sc:512 * sc + 512],
                                start=(ko == 0), stop=(ko == KO - 1),
                            )
                    for i in range(8):
                        nt, sc = (0, 2, 1, 3)[2 * rnd + i // 4], i % 4
                        nc.scalar.activation(
                            qk_sb[:, nt, 512 * sc:512 * sc + 512], ps8[i],
                            mybir.ActivationFunctionType.Identity,
                            bias=bqk_sb[:, nt:nt + 1], scale=1.0,
                        )

                # v natural: out[s-tile, (h,hd)] = hidden @ wv.
                # 2-tile rounds: first-fit slot reuse keeps v cycling in the
                # low PSUM slots, so the other 4 banks free up as soon as the
                # qk rounds drain -- letting head-0 scores/exp (whose pool
                # aliases those banks) start while v is still running.
                for rnd in range(8):
                    ps2 = [p1ps.tile([128, 512], DT_F, tag="g", name=f"v{rnd}{i}")
                           for i in range(2)]
                    for ko in range(KO):
                        for i in range(2):
                            st = 2 * rnd + i
                            nc.tensor.matmul(
                                ps2[i][:, 0:256],
                                hT[:, ko, 128 * st:128 * st + 128],
                                w_sb[:, ko, 512:768],
                                start=(ko == 0), stop=(ko == KO - 1),
                            )
                    for i in range(2):
                        st = 2 * rnd + i
                        nc.vector.tensor_add(
                            out=v_sb[:, st, :, 0:64],
                            in0=ps2[i][:, 0:256].rearrange("p (h d) -> p h d", d=64),
                            in1=bv_sb.rearrange("p (h d) -> p h d", d=64),
                        )

            # ================= phase 2: attention per head =================
            with tc.tile_pool(name="p2sb", bufs=8) as p2sb, \
                 tc.tile_pool(name="p2cp", bufs=8) as p2cp, \
                 tc.tile_pool(name="p2rep", bufs=6) as p2rep, \
                 tc.tile_pool(name="p2row", bufs=6) as p2row, \
                 tc.tile_pool(name="p2sc", bufs=2, space="PSUM") as p2sc, \
                 tc.tile_pool(name="p2av", bufs=4, space="PSUM") as p2av:
                # global piece list across heads: the depth-2 software
                # pipeline runs straight through head boundaries, so the
                # next head's scores are already in flight while the
                # previous head's tail (av matmuls + drains) executes.
                all_pieces = []
                for lh in range(HPC):
                    for t in range(NT):
                        for p in range(t // 8, 2):
                            gs = max(1024 * p, 128 * t)
                            all_pieces.append((lh, t, gs, 1024 * (p + 1) - gs))
                av_ps_by = {}

                def drain_chunk(lh, c):
                    # av fully accumulated: copy to SBUF (frees the PSUM
                    # bank), reciprocal of the denominator row, DRAM-bounce
                    # partition-broadcast, normalize into ao2 (GPSIMD:
                    # all-SBUF operands, keeps DVE off the critical path).
                    bp = 64 * (lh % 2)
                    cp = p2cp.tile([65, 512], DT_F, tag="avcp",
                                   name=f"cp{lh}{c}")
                    nc.vector.tensor_copy(cp, av_ps_by[lh][c][0:65, :])
                    rec = p2row.tile([1, 512], DT_F, tag="rec")
                    nc.vector.reciprocal(rec, cp[64:65, :])
                    dtile = dram.tile([1, 512], DT_F)
                    nc.gpsimd.dma_start(dtile, rec)
                    rep = p2rep.tile([64, 512], DT_F, tag="rep")
                    srcap = dtile[0, :]
                    bcast = bass.AP(
                        tensor=srcap.tensor, offset=srcap.offset,
                        ap=[[0, 64]] + [list(pr) for pr in srcap.ap],
                    )
                    nc.gpsimd.dma_start(rep, bcast)
                    nc.gpsimd.tensor_mul(
                        out=ao2[bp:bp + 64, lh // 2, 512 * c:512 * (c + 1)],
                        in0=cp[0:64, :],
                        in1=rep,
                    )

                def emit_tail(lh, t, gs, width, at_sb):
                    # exp consumers for an already-scored piece: causal 0/1
                    # mask on the diagonal block (GPSIMD, all-SBUF, never
                    # gates ACT) + out.T/denom accumulation.
                    if gs == 128 * t:
                        nc.gpsimd.tensor_mul(
                            out=at_sb[:, 0:128], in0=at_sb[:, 0:128],
                            in1=dm_sb,
                        )
                    v_aug = v_sb[:, t, lh, :]
                    off = 0
                    while off < width:
                        g0 = gs + off
                        c = g0 // 512
                        w512 = min(512, 512 * (c + 1) - g0)
                        t_last = min(NT - 1, 4 * c + 3)
                        nc.tensor.matmul(
                            av_ps_by[lh][c][0:65, (g0 % 512):(g0 % 512) + w512],
                            v_aug,
                            at_sb[:, off:off + w512],
                            start=(t == 0), stop=(t == t_last),
                        )
                        off += w512
                    # chunk t//4 fully accumulated after the last piece of
                    # t in (3, 7, 11, 15)
                    if gs + width == 2048 and t % 4 == 3:
                        drain_chunk(lh, t // 4)

                pending = []
                for lh, t, gs, width in all_pieces:
                    bp = 64 * (lh % 2)
                    q_nt = lh // 2
                    k_nt = 2 + lh // 2
                    if t == 0 and gs == 0:
                        av_ps_by[lh] = [
                            p2av.tile([128, 512], DT_F, tag="av",
                                      name=f"av{lh}{c}")
                            for c in range(4)
                        ]
                    lhsT_k = qk_sb[bp:bp + 64, k_nt, 128 * t:128 * t + 128]
                    sc_ps = p2sc.tile([128, 1024], DT_F, tag="sc")
                    off = 0
                    while off < width:
                        w512 = min(512, width - off)
                        nc.tensor.matmul(
                            sc_ps[:, off:off + w512],
                            lhsT_k,
                            qk_sb[bp:bp + 64, q_nt, gs + off:gs + off + w512],
                            start=True, stop=True,
                        )
                        off += w512
                    at_sb = p2sb.tile([128, 1024], DT_R, tag="attnT")
                    nc.scalar.activation(
                        at_sb[:, :width], sc_ps[:, :width],
                        mybir.ActivationFunctionType.Exp,
                        bias=lm_sb[:, lh, t:t + 1], scale=0.125,
                    )
                    pending.append((lh, t, gs, width, at_sb))
                    if len(pending) > 4:
                        emit_tail(*pending.pop(0))
                for pc in pending:
                    emit_tail(*pc)

            # ================= phase 3: c_proj partial =================
            with tc.tile_pool(name="p3sb", bufs=6) as p3sb, \
                 tc.tile_pool(name="p3ps", bufs=4, space="PSUM") as p3ps:
                for st in range(NT):
                    for ec in range(2):
                        ps = p3ps.tile([128, 512], DT_F, tag="pr")
                        for j in range(2):
                            nc.tensor.matmul(
                                ps,
                                ao2[:, j, 128 * st:128 * st + 128],
                                pw_sb[:, j, 512 * ec:512 * ec + 512],
                                start=(j == 0), stop=(j == 1),
                            )
                        o_sb = p3sb.tile([128, 512], DT_F, tag="out")
                        k = 2 * st + ec
                        if k % 3 == 0:
                            nc.scalar.copy(o_sb, ps)
                        else:
                            nc.vector.tensor_copy(o_sb, ps)
                        oq = (nc.scalar, nc.sync, nc.sync)[k % 3]
                        oq.dma_start(
                            out[128 * st:128 * st + 128, 512 * ec:512 * ec + 512],
                            o_sb,
                        )
    return nc


_NC = None


def _get_nc():
    global _NC
    if _NC is None:
        _NC = build_program()
    return _NC


# ---------------------------------------------------------------- host prep

def make_in_maps(hidden_states, c_attn_w, c_attn_b, c_proj_w):
    first_end = S // 3
    second_end = 2 * S // 3
    pos = np.arange(S)
    regions = [pos < first_end,
               (pos >= first_end) & (pos < second_end),
               pos >= second_end]
    mult = np.ones((H, S), dtype=np.float64)
    for h, r in HEAD_REGION.items():
        mult[h] = 1.0 + (FOCUS - 1.0) * regions[r].astype(np.float64)
    logm = np.log(mult).astype(np.float32)  # [H, S]

    p = np.arange(128)[:, None]
    j = np.arange(128)[None, :]
    diag = (j >= p).astype(np.float32)  # 0/1 keep-mask, applied post-exp

    in_maps = []
    for c in range(NCORES):
        b, g = divmod(c, GROUPS)
        h0 = HPC * g
        cs = slice(256 * g, 256 * g + 256)
        w_qkv = np.concatenate(
            [c_attn_w[:, cs], c_attn_w[:, 1024:2048][:, cs],
             c_attn_w[:, 2048:3072][:, cs]], axis=1,
        ).astype(np.float32)
        bqk = np.concatenate(
            [c_attn_b[cs], c_attn_b[1024:2048][cs]]
        ).reshape(4, 128).T.copy().astype(np.float32)
        bv = np.broadcast_to(
            c_attn_b[2048:3072][cs], (128, 256)
        ).astype(np.float32).copy()
        # pw2[p, j, e]: head pair j=(2j, 2j+1); p<64 -> head 2j row p,
        # p>=64 -> head 2j+1 row p-64  (matches ao2 partition interleave)
        pw = c_proj_w[64 * h0:64 * h0 + 256, :].reshape(2, 128, D)
        pw = np.ascontiguousarray(pw.transpose(1, 0, 2)).astype(np.float32)
        lm = logm[h0:h0 + HPC].reshape(HPC, S // 128, 128)
        lm = np.ascontiguousarray(lm.transpose(2, 0, 1)).astype(np.float32)
        in_maps.append({
            "hiddenT": np.ascontiguousarray(hidden_states[b].T).astype(np.float32),
            "w_qkv": w_qkv,
            "bqk": bqk,
            "bv_rep": bv,
            "projw": pw,
            "diag_mask": diag,
            "logmult": lm,
        })
    return in_maps


def run_cores(in_maps, trace=False, **kw):
    from concourse.bass_utils import run_bass_kernel_spmd
    nc = _get_nc()
    return run_bass_kernel_spmd(nc, in_maps, core_ids=list(range(NCORES)),
                                trace=trace, **kw)


def kernel(hidden_states, c_attn_w, c_attn_b, c_proj_w, c_proj_b):
    hidden_states = np.asarray(hidden_states, dtype=np.float32)
    c_attn_w = np.asarray(c_attn_w, dtype=np.float32)
    c_attn_b = np.asarray(c_attn_b, dtype=np.float32)
    c_proj_w = np.asarray(c_proj_w, dtype=np.float32)
    c_proj_b = np.asarray(c_proj_b, dtype=np.float32)

    in_maps = make_in_maps(hidden_states, c_attn_w, c_attn_b, c_proj_w)
    res = run_cores(in_maps)
    out = np.zeros((B, S, D), dtype=np.float32)
    for c in range(NCORES):
        out[c // GROUPS] += res.results[c]["out"]
    out += c_proj_b[None, None, :]
    return out



# revision 3
# speedup vs baseline: 1.1259x; 1.1259x over previous
"""DivergentAttention Trainium2 kernel (8 NeuronCores, Bass/Tile).

Problem: GPT-2 style causal self-attention (B=2, S=2048, D=1024, H=16,
hd=64) where heads 0/1/2 re-weight their attention toward a token region
(first/middle/last third of the sequence) with factor 1.6 and renormalize.

Key identity: softmax(s)*m / sum(softmax(s)*m) == softmax(s + log m), so the
per-head region reweight folds into an additive per-(head, key-position)
bias on the scores -- no second normalization pass needed.  Scores are small
(|s|<~5) so the max-subtraction pass is skipped entirely.

Sharding: tensor-parallel over (batch, head-group): core c handles batch
c//4 and heads [4*(c%4), 4*(c%4)+4).  Each core computes the QKV projection
for its 4 heads, full causal attention, and its partial c_proj; the host
sums the 8 partials (fp32) and adds c_proj_b.

All matmul inputs are bf16 (error budget allows it; bf16 runs the PE at one
row per output column for every width, unlike fp32r which needs N>=256).

Attention structure per head (the big change vs the v1 kernel):
  - scoresT [sk-tile 128, q] = kT.T @ qT as before, exp'd (ScalarE, scale
    1/8, bias log mult) into a per-head PERSISTENT attn buffer at_sb
    [128, 17408] bf16 holding the whole causal triangle.
  - AV is FLIPPED: out[q-tile 128, 65] = attnT(stationary).T @ v_aug --
    65 output columns per (q-tile, sk-tile) pair instead of 512-wide
    rows, halving AV PE time.  v_aug column 64 is ones, so column 64 of
    the output is the softmax denominator per q ROW -- normalization
    becomes a per-partition scalar multiply (DVE reciprocal + mult), no
    partition-broadcast DMA bounce.
  - The normalized per-head output O' [q, 64] bf16 is transposed back to
    [64*(h%2)+d, q] for c_proj with a PE transpose (identity moving
    operand), 128 rows per q-tile.
  - c_proj: two K=128 matmuls per output tile against pw2 [128, 2, D].

Window schedule (PE-balance: exp on ScalarE is the per-head pacing limit,
so PE filler work is spread into the exp-bound windows):
  w0: head 0 + one v-projection tile per round (pr-tag PSUM).
  w1: head 1 + the deferred qk projection round for heads 2/3 (8 groups,
      rounds 0..7) + pair-0 transposes.
  w2: head 3 + head-2 score tiles 0..4 pulled ahead (rounds 11..15).
  w3: head 2 (scores 5..15) + pair-1 transposes + c_proj, software-
      pipelined (transpose at round t-2, c_proj at round t-3) so the PE
      never waits on the DVE/ScalarE drains.
"""

import numpy as np
import ml_dtypes

import concourse.bass as bass
import concourse.tile as tile
from concourse import mybir
from concourse import bass_utils, bass2jax

# ---------------------------------------------------------------- constants
B, S, D, H, HD = 2, 2048, 1024, 16, 64
NCORES = 8
HPC = 4              # heads per core
GROUPS = 4           # head groups
FOCUS = 1.6
HEAD_REGION = {0: 0, 1: 1, 2: 2}
NT = S // 128        # 16 sk/q tiles
KO = D // 128        # 8 contraction chunks
BF = mybir.dt.bfloat16
F32 = mybir.dt.float32

# column offset of tile t's rows inside the per-head attn buffer
OFF = [0] * (NT + 1)
for _t in range(NT):
    OFF[_t + 1] = OFF[_t] + (S - 128 * _t)
ATW = OFF[NT]        # 17408

# ------------------------------------------------- walrus multi-wait fixup
# This container's walrus accepts only ONE sync-wait per TPB instruction,
# but Tile attaches one wait per dependency proc.  Rewrite the BIR JSON just
# before walrus: hoist all-but-one wait of a multi-wait instruction onto
# standalone same-engine NoOps inserted immediately before it (same-engine
# program order is preserved, so semantics are unchanged).
try:
    import orjson as _json
except ImportError:  # pragma: no cover
    import json as _json

_orig_compile_bir_kernel = bass_utils.compile_bir_kernel
_wfix_counter = [0]


def _fix_bir(bir_json):
    d = _json.loads(bir_json)
    changed = False
    for fn in d.get("functions", []):
        for blk in fn.get("blocks", []):
            out = []
            for inst in blk.get("instructions", []):
                si = inst.get("sync_info")
                if si:
                    waits = si.get("on_wait") or []
                    if len(waits) > 1:
                        changed = True
                        for w in waits[:-1]:
                            _wfix_counter[0] += 1
                            nop = {
                                "engine": inst["engine"],
                                "ins": [],
                                "name": f"I-wfix-{_wfix_counter[0]}",
                                "opcode": "NoOp",
                                "outs": [],
                                "sync_info": {"on_update": [], "on_wait": [w]},
                            }
                            if "debug" in inst:
                                nop["debug"] = inst["debug"]
                            out.append(nop)
                        si["on_wait"] = waits[-1:]
                out.append(inst)
            blk["instructions"] = out
    return _json.dumps(d) if changed else bir_json


def _patched_compile_bir_kernel(bir_json, tmpdir, neff_name="file.neff"):
    return _orig_compile_bir_kernel(_fix_bir(bir_json), tmpdir, neff_name=neff_name)


def _install_waitfix():
    bass_utils.compile_bir_kernel = _patched_compile_bir_kernel
    bass2jax.compile_bir_kernel = _patched_compile_bir_kernel


_install_waitfix()

# ---------------------------------------------------------------- program


def build_program():
    """One SPMD Bass program; per-core differences come in via inputs."""
    nc = bass.Bass()

    hiddenT = nc.dram_tensor("hiddenT", [D, S], BF, kind="ExternalInput")
    w_qkv = nc.dram_tensor("w_qkv", [D, 768], BF, kind="ExternalInput")
    bqk = nc.dram_tensor("bqk", [128, 4], F32, kind="ExternalInput")
    bv_rep = nc.dram_tensor("bv_rep", [128, 256], F32, kind="ExternalInput")
    projw = nc.dram_tensor("projw", [128, 2, D], BF, kind="ExternalInput")
    diag_mask = nc.dram_tensor("diag_mask", [128, 128], BF, kind="ExternalInput")
    ident = nc.dram_tensor("ident", [128, 128], BF, kind="ExternalInput")
    logmult = nc.dram_tensor("logmult", [128, HPC, NT], F32, kind="ExternalInput")
    out = nc.dram_tensor("out", [S, D], BF, kind="ExternalOutput")

    with tile.TileContext(nc) as tc:
        with tc.tile_pool(name="persist", bufs=1) as persist, \
             tc.tile_pool(name="atp", bufs=2) as atp, \
             tc.tile_pool(name="stgp", bufs=2) as stgp, \
             tc.tile_pool(name="recp", bufs=4) as recp, \
             tc.tile_pool(name="outp", bufs=4) as outp:

            # ---- persistent SBUF ----
            hT = persist.tile([128, KO, S], BF)           # 4 MB
            w_sb = persist.tile([128, KO, 768], BF)       # 1.5 MB
            qk_sb = persist.tile([128, 4, S], BF)         # 2 MB
            v_sb = persist.tile([128, NT, HPC, 65], BF)   # ~1 MB
            ao2 = persist.tile([128, 2, S], BF)           # attn_outT, 1 MB
            bqk_sb = persist.tile([128, 4], F32)
            bv_sb = persist.tile([128, 256], F32)
            pw_sb = persist.tile([128, 2, D], BF)         # 0.5 MB
            dm_sb = persist.tile([128, 128], BF)
            id_sb = persist.tile([128, 128], BF)
            lm_sb = persist.tile([128, HPC, NT], F32)

            nc.sync.dma_start(bqk_sb, bqk[:, :])
            nc.vector.memset(v_sb[:, :, :, 64:65], 1.0)

            # input loads: hT alternates the SP and ACT HWDGE queues (ACT
            # is idle until the first exp), w + small tensors ride the
            # GPSIMD SWDGE queue.
            hT_src = hiddenT.rearrange("(ko p) s -> p ko s", p=128)
            w_src = w_qkv.rearrange("(ko p) n -> p ko n", p=128)
            for ko in range(KO):
                q = nc.sync if ko % 2 == 0 else nc.scalar
                q.dma_start(hT[:, ko, :], hT_src[:, ko, :])
                nc.gpsimd.dma_start(w_sb[:, ko, :], w_src[:, ko, :])
                if ko == 0:
                    nc.sync.dma_start(bv_sb, bv_rep[:, :])
                    nc.gpsimd.dma_start(dm_sb, diag_mask[:, :])
                    nc.gpsimd.dma_start(id_sb, ident[:, :])
                    nc.gpsimd.dma_start(lm_sb, logmult[:, :, :])
                    nc.gpsimd.dma_start(pw_sb, projw[:, :, :])

            # ========== qk projection round 0 (heads 0/1: nt 0 and 2) =====
            # ko (contraction) outer, 8 resident PSUM groups: PE consumes
            # each hiddenT/w chunk as it arrives from HBM.
            with tc.tile_pool(name="p1ps", bufs=8, space="PSUM") as p1ps:
                ps8 = [p1ps.tile([128, 512], F32, tag="g", name=f"q0{i}")
                       for i in range(8)]
                for ko in range(KO):
                    for i in range(8):
                        nt, sc = (0, 2)[i // 4], i % 4
                        nc.tensor.matmul(
                            ps8[i],
                            w_sb[:, ko, 128 * nt:128 * nt + 128],
                            hT[:, ko, 512 * sc:512 * sc + 512],
                            start=(ko == 0), stop=(ko == KO - 1),
                        )
                for i in range(8):
                    nt, sc = (0, 2)[i // 4], i % 4
                    nc.vector.tensor_scalar_add(
                        qk_sb[:, nt, 512 * sc:512 * sc + 512], ps8[i],
                        bqk_sb[:, nt:nt + 1],
                    )

            # ================= attention + everything else =================
            with tc.tile_pool(name="psc", bufs=2, space="PSUM") as psc, \
                 tc.tile_pool(name="pav", bufs=2, space="PSUM") as pav, \
                 tc.tile_pool(name="ppr", bufs=2, space="PSUM") as ppr:

                stg_by = {}

                def emit_scores(lh, t):
                    """score pieces + exp into at bufs, then diag mask."""
                    bp = 64 * (lh % 2)
                    qn, kn = lh // 2, 2 + lh // 2
                    at_sb = at_by[lh]
                    gs = 128 * t
                    while gs < S:
                        w = min(1024, S - gs)
                        sc = psc.tile([128, 1024], F32, tag="sc")
                        o = 0
                        while o < w:
                            n = min(512, w - o)
                            nc.tensor.matmul(
                                sc[:, o:o + n],
                                qk_sb[bp:bp + 64, kn, 128 * t:128 * t + 128],
                                qk_sb[bp:bp + 64, qn, gs + o:gs + o + n],
                                start=True, stop=True,
                            )
                            o += n
                        nc.scalar.activation(
                            at_sb[:, OFF[t] + gs - 128 * t:
                                  OFF[t] + gs - 128 * t + w],
                            sc[:, :w],
                            mybir.ActivationFunctionType.Exp,
                            bias=lm_sb[:, lh, t:t + 1], scale=0.125,
                        )
                        gs += w
                    # causal 0/1 mask on the diagonal block (GPSIMD,
                    # all-SBUF, never gates ScalarE).
                    nc.gpsimd.tensor_mul(
                        out=at_by[lh][:, OFF[t]:OFF[t] + 128],
                        in0=at_by[lh][:, OFF[t]:OFF[t] + 128],
                        in1=dm_sb,
                    )

                def emit_av(lh, t):
                    """flipped AV for q-tile t + per-partition normalize."""
                    at_sb = at_by[lh]
                    av = pav.tile([128, 65], F32, tag="av", name=f"av{lh}{t}")
                    for u in range(t + 1):
                        nc.tensor.matmul(
                            av,
                            at_sb[:, OFF[u] + 128 * (t - u):
                                  OFF[u] + 128 * (t - u) + 128],
                            v_sb[:, u, lh, :],
                            start=(u == 0), stop=(u == t),
                        )
                    rec = recp.tile([128, 1], F32, tag="rec")
                    nc.vector.reciprocal(rec, av[:, 64:65])
                    nc.vector.tensor_scalar_mul(
                        stg_by[lh // 2][:, t, lh % 2, :], av[:, 0:64], rec,
                    )

                def emit_transpose(j, st):
                    """O' [q,(e,d)] block -> ao2 [(e,d), q] via PE."""
                    tp = pav.tile([128, 128], BF, tag="av", name=f"tp{j}{st}")
                    nc.tensor.matmul(
                        tp, stg_by[j][:, st, :, :], id_sb,
                        start=True, stop=True, is_transpose=True,
                    )
                    nc.vector.tensor_copy(
                        ao2[:, j, 128 * st:128 * st + 128], tp,
                    )

                def emit_cproj(st):
                    for ec in range(2):
                        pr = ppr.tile([128, 512], F32, tag="pr")
                        for jj in range(2):
                            nc.tensor.matmul(
                                pr,
                                ao2[:, jj, 128 * st:128 * st + 128],
                                pw_sb[:, jj, 512 * ec:512 * ec + 512],
                                start=(jj == 0), stop=(jj == 1),
                            )
                        o_sb = outp.tile([128, 512], BF, tag="osb")
                        if ec == 0:
                            nc.vector.tensor_copy(o_sb, pr)
                        else:
                            nc.scalar.copy(o_sb, pr)
                        nc.sync.dma_start(
                            out[128 * st:128 * st + 128,
                                512 * ec:512 * ec + 512],
                            o_sb,
                        )

                def emit_vround(st):
                    """v natural: out[s-tile, (h,d)] = hidden @ wv."""
                    ps = ppr.tile([128, 512], F32, tag="pr", name=f"v{st}")
                    for ko in range(KO):
                        nc.tensor.matmul(
                            ps[:, 0:256],
                            hT[:, ko, 128 * st:128 * st + 128],
                            w_sb[:, ko, 512:768],
                            start=(ko == 0), stop=(ko == KO - 1),
                        )
                    nc.vector.tensor_add(
                        out=v_sb[:, st, :, 0:64],
                        in0=ps[:, 0:256].rearrange("p (h d) -> p h d", d=64),
                        in1=bv_sb.rearrange("p (h d) -> p h d", d=64),
                    )

                def emit_qkr1(g):
                    """deferred qk projection for heads 2/3 (nt 1 and 3)."""
                    nt, sc4 = (1, 3)[g // 4], g % 4
                    ps = ppr.tile([128, 512], F32, tag="pr", name=f"qr{g}")
                    for ko in range(KO):
                        nc.tensor.matmul(
                            ps,
                            w_sb[:, ko, 128 * nt:128 * nt + 128],
                            hT[:, ko, 512 * sc4:512 * sc4 + 512],
                            start=(ko == 0), stop=(ko == KO - 1),
                        )
                    nc.vector.tensor_scalar_add(
                        qk_sb[:, nt, 512 * sc4:512 * sc4 + 512], ps,
                        bqk_sb[:, nt:nt + 1],
                    )

                at_by = {}

                def new_at(lh):
                    at_by[lh] = atp.tile([128, ATW], BF, tag="at",
                                         name=f"at{lh}")
                    if lh // 2 not in stg_by:
                        stg_by[lh // 2] = stgp.tile([128, NT, 2, 64], BF,
                                                    tag="stg",
                                                    name=f"stg{lh // 2}")

                # ---- w0: head 0, v-projection rounds as PE filler ----
                new_at(0)
                for t in range(NT):
                    emit_vround(t)
                    emit_scores(0, t)
                    if t >= 1:
                        emit_av(0, t - 1)
                emit_av(0, NT - 1)

                # ---- w1: head 1, deferred qk round + pair-0 transposes ----
                new_at(1)
                for t in range(NT):
                    if t < 8:
                        emit_qkr1(t)
                    emit_scores(1, t)
                    if t >= 1:
                        emit_av(1, t - 1)
                    if t >= 2:
                        emit_transpose(0, t - 2)
                emit_av(1, NT - 1)
                emit_transpose(0, NT - 2)
                emit_transpose(0, NT - 1)

                # ---- w2: head 3, head-2 scores 0..4 pulled ahead ----
                new_at(3)
                for t in range(NT):
                    emit_scores(3, t)
                    if t >= 1:
                        emit_av(3, t - 1)
                    if t >= 11:
                        if t == 11:
                            new_at(2)
                        emit_scores(2, t - 11)
                emit_av(3, NT - 1)

                # ---- w3: head 2 rest + pair-1 transposes + c_proj ----
                for t in range(NT):
                    if t >= 5:
                        emit_scores(2, t)
                    if t >= 1:
                        emit_av(2, t - 1)
                    if t >= 2:
                        emit_transpose(1, t - 2)
                    if t >= 3:
                        emit_cproj(t - 3)
                emit_av(2, NT - 1)
                emit_transpose(1, NT - 2)
                emit_transpose(1, NT - 1)
                for st in (NT - 3, NT - 2, NT - 1):
                    emit_cproj(st)
    return nc


_NC = None


def _get_nc():
    global _NC
    if _NC is None:
        _NC = build_program()
    return _NC


# ---------------------------------------------------------------- host prep

def make_in_maps(hidden_states, c_attn_w, c_attn_b, c_proj_w):
    first_end = S // 3
    second_end = 2 * S // 3
    pos = np.arange(S)
    regions = [pos < first_end,
               (pos >= first_end) & (pos < second_end),
               pos >= second_end]
    mult = np.ones((H, S), dtype=np.float64)
    for h, r in HEAD_REGION.items():
        mult[h] = 1.0 + (FOCUS - 1.0) * regions[r].astype(np.float64)
    logm = np.log(mult).astype(np.float32)  # [H, S]

    p = np.arange(128)[:, None]
    j = np.arange(128)[None, :]
    diag = (j >= p).astype(np.float32)  # 0/1 keep-mask, applied post-exp
    iden = (j == p).astype(np.float32)

    bf = ml_dtypes.bfloat16
    in_maps = []
    for c in range(NCORES):
        b, g = divmod(c, GROUPS)
        h0 = HPC * g
        cs = slice(256 * g, 256 * g + 256)
        w_qkv = np.concatenate(
            [c_attn_w[:, cs], c_attn_w[:, 1024:2048][:, cs],
             c_attn_w[:, 2048:3072][:, cs]], axis=1,
        ).astype(bf)
        bqk_h = np.concatenate(
            [c_attn_b[cs], c_attn_b[1024:2048][cs]]
        ).reshape(4, 128).T.copy().astype(np.float32)
        bv = np.broadcast_to(
            c_attn_b[2048:3072][cs], (128, 256)
        ).astype(np.float32).copy()
        # pw2[p, j, e]: head pair j=(2j, 2j+1); p<64 -> head 2j row p,
        # p>=64 -> head 2j+1 row p-64  (matches ao2 partition interleave)
        pw = c_proj_w[64 * h0:64 * h0 + 256, :].reshape(2, 128, D)
        pw = np.ascontiguousarray(pw.transpose(1, 0, 2)).astype(bf)
        lm = logm[h0:h0 + HPC].reshape(HPC, S // 128, 128)
        lm = np.ascontiguousarray(lm.transpose(2, 0, 1)).astype(np.float32)
        in_maps.append({
            "hiddenT": np.ascontiguousarray(hidden_states[b].T).astype(bf),
            "w_qkv": w_qkv,
            "bqk": bqk_h,
            "bv_rep": bv,
            "projw": pw,
            "diag_mask": diag.astype(bf),
            "ident": iden.astype(bf),
            "logmult": lm,
        })
    return in_maps


def run_cores(in_maps, trace=False, **kw):
    from concourse.bass_utils import run_bass_kernel_spmd
    nc = _get_nc()
    return run_bass_kernel_spmd(nc, in_maps, core_ids=list(range(NCORES)),
                                trace=trace, **kw)


def kernel(hidden_states, c_attn_w, c_attn_b, c_proj_w, c_proj_b):
    hidden_states = np.asarray(hidden_states, dtype=np.float32)
    c_attn_w = np.asarray(c_attn_w, dtype=np.float32)
    c_attn_b = np.asarray(c_attn_b, dtype=np.float32)
    c_proj_w = np.asarray(c_proj_w, dtype=np.float32)
    c_proj_b = np.asarray(c_proj_b, dtype=np.float32)

    in_maps = make_in_maps(hidden_states, c_attn_w, c_attn_b, c_proj_w)
    res = run_cores(in_maps)
    out = np.zeros((B, S, D), dtype=np.float32)
    for c in range(NCORES):
        out[c // GROUPS] += np.asarray(res.results[c]["out"],
                                       dtype=np.float32)
    out += c_proj_b[None, None, :]
    return out


# revision 6
# speedup vs baseline: 1.1553x; 1.0261x over previous
"""DivergentAttention Trainium2 kernel (8 NeuronCores, Bass/Tile).

Problem: GPT-2 style causal self-attention (B=2, S=2048, D=1024, H=16,
hd=64) where heads 0/1/2 re-weight their attention toward a token region
(first/middle/last third of the sequence) with factor 1.6 and renormalize.

Key identity: softmax(s)*m / sum(softmax(s)*m) == softmax(s + log m), so the
per-head region reweight folds into an additive per-(head, key-position)
bias on the scores -- no second normalization pass needed.  Scores are small
(|s|<~5) so the max-subtraction pass is skipped entirely.

Sharding: tensor-parallel over (batch, head-group): core c handles batch
c//4 and heads [4*(c%4), 4*(c%4)+4).  Each core computes the QKV projection
for its 4 heads, full causal attention, and its partial c_proj; the host
sums the 8 partials (fp32) and adds c_proj_b.

All matmul inputs are bf16 (error budget allows it; bf16 runs the PE at one
row per output column for every width, unlike fp32r which needs N>=256).

Attention structure per head (the big change vs the v1 kernel):
  - scoresT [sk-tile 128, q] = kT.T @ qT as before, exp'd (ScalarE, scale
    1/8, bias log mult) into a per-head PERSISTENT attn buffer at_sb
    [128, 17408] bf16 holding the whole causal triangle.
  - AV is FLIPPED: out[q-tile 128, 65] = attnT(stationary).T @ v_aug --
    65 output columns per (q-tile, sk-tile) pair instead of 512-wide
    rows, halving AV PE time.  v_aug column 64 is ones, so column 64 of
    the output is the softmax denominator per q ROW -- normalization
    becomes a per-partition scalar multiply (DVE reciprocal + mult), no
    partition-broadcast DMA bounce.
  - The normalized per-head output O' [q, 64] bf16 is transposed back to
    [64*(h%2)+d, q] for c_proj with a PE transpose (identity moving
    operand), 128 rows per q-tile.
  - c_proj: two K=128 matmuls per output tile against pw2 [128, 2, D].

Window schedule (PE-balance: exp on ScalarE is the per-head pacing limit,
so PE filler work is spread into the exp-bound windows):
  w0: head 0 + one v-projection tile per round (pr-tag PSUM).
  w1: head 1 + the deferred qk projection round for heads 2/3 (8 groups,
      rounds 0..7) + pair-0 transposes.
  w2: head 3 + head-2 score tiles 0..4 pulled ahead (rounds 11..15).
  w3: head 2 (scores 5..15) + pair-1 transposes + c_proj, software-
      pipelined (transpose at round t-2, c_proj at round t-3) so the PE
      never waits on the DVE/ScalarE drains.
"""

import numpy as np
import ml_dtypes

import concourse.bass as bass
import concourse.tile as tile
from concourse import mybir
from concourse import bass_utils, bass2jax

# ---------------------------------------------------------------- constants
B, S, D, H, HD = 2, 2048, 1024, 16, 64
NCORES = 8
HPC = 4              # heads per core
GROUPS = 4           # head groups
FOCUS = 1.6
HEAD_REGION = {0: 0, 1: 1, 2: 2}
NT = S // 128        # 16 sk/q tiles
KO = D // 128        # 8 contraction chunks
BF = mybir.dt.bfloat16
F32 = mybir.dt.float32

# column offset of tile t's rows inside the per-head attn buffer
OFF = [0] * (NT + 1)
for _t in range(NT):
    OFF[_t + 1] = OFF[_t] + (S - 128 * _t)
ATW = OFF[NT]        # 17408

# ------------------------------------------------- walrus multi-wait fixup
# This container's walrus accepts only ONE sync-wait per TPB instruction,
# but Tile attaches one wait per dependency proc.  Rewrite the BIR JSON just
# before walrus: hoist all-but-one wait of a multi-wait instruction onto
# standalone same-engine NoOps inserted immediately before it (same-engine
# program order is preserved, so semantics are unchanged).
try:
    import orjson as _json
except ImportError:  # pragma: no cover
    import json as _json

_orig_compile_bir_kernel = bass_utils.compile_bir_kernel
_wfix_counter = [0]


def _fix_bir(bir_json):
    d = _json.loads(bir_json)
    changed = False
    for fn in d.get("functions", []):
        for blk in fn.get("blocks", []):
            out = []
            for inst in blk.get("instructions", []):
                si = inst.get("sync_info")
                if si:
                    waits = si.get("on_wait") or []
                    if len(waits) > 1:
                        changed = True
                        for w in waits[:-1]:
                            _wfix_counter[0] += 1
                            nop = {
                                "engine": inst["engine"],
                                "ins": [],
                                "name": f"I-wfix-{_wfix_counter[0]}",
                                "opcode": "NoOp",
                                "outs": [],
                                "sync_info": {"on_update": [], "on_wait": [w]},
                            }
                            if "debug" in inst:
                                nop["debug"] = inst["debug"]
                            out.append(nop)
                        si["on_wait"] = waits[-1:]
                out.append(inst)
            blk["instructions"] = out
    return _json.dumps(d) if changed else bir_json


def _patched_compile_bir_kernel(bir_json, tmpdir, neff_name="file.neff"):
    return _orig_compile_bir_kernel(_fix_bir(bir_json), tmpdir, neff_name=neff_name)


def _install_waitfix():
    bass_utils.compile_bir_kernel = _patched_compile_bir_kernel
    bass2jax.compile_bir_kernel = _patched_compile_bir_kernel


_install_waitfix()

# ---------------------------------------------------------------- program


def build_program():
    """One SPMD Bass program; per-core differences come in via inputs."""
    nc = bass.Bass()

    hiddenT = nc.dram_tensor("hiddenT", [D, S], BF, kind="ExternalInput")
    w_qkv = nc.dram_tensor("w_qkv", [D, 768], BF, kind="ExternalInput")
    bqk = nc.dram_tensor("bqk", [128, 4], F32, kind="ExternalInput")
    bv_rep = nc.dram_tensor("bv_rep", [128, 256], F32, kind="ExternalInput")
    projw = nc.dram_tensor("projw", [128, 2, D], BF, kind="ExternalInput")
    diag_mask = nc.dram_tensor("diag_mask", [128, 128], BF, kind="ExternalInput")
    ident = nc.dram_tensor("ident", [128, 128], BF, kind="ExternalInput")
    logmult = nc.dram_tensor("logmult", [128, HPC, NT], F32, kind="ExternalInput")
    out = nc.dram_tensor("out", [S, D], BF, kind="ExternalOutput")

    with tile.TileContext(nc) as tc:
        with tc.tile_pool(name="persist", bufs=1) as persist, \
             tc.tile_pool(name="atp", bufs=2) as atp, \
             tc.tile_pool(name="stgp", bufs=2) as stgp, \
             tc.tile_pool(name="recp", bufs=4) as recp, \
             tc.tile_pool(name="outp", bufs=4) as outp:

            # ---- persistent SBUF ----
            hT = persist.tile([128, KO, S], BF)           # 4 MB
            w_sb = persist.tile([128, KO, 768], BF)       # 1.5 MB
            qk_sb = persist.tile([128, 4, S], BF)         # 2 MB
            v_sb = persist.tile([128, NT, HPC, 65], BF)   # ~1 MB
            ao2 = persist.tile([128, 2, S], BF)           # attn_outT, 1 MB
            bqk_sb = persist.tile([128, 4], F32)
            bv_sb = persist.tile([128, 256], F32)
            pw_sb = persist.tile([128, 2, D], BF)         # 0.5 MB
            dm_sb = persist.tile([128, 128], BF)
            id_sb = persist.tile([128, 128], BF)
            lm_sb = persist.tile([128, HPC, NT], F32)

            nc.sync.dma_start(bqk_sb, bqk[:, :])
            nc.vector.memset(v_sb[:, :, :, 64:65], 1.0)

            # input loads: hT alternates the SP and ACT HWDGE queues (ACT
            # is idle until the first exp), w + small tensors ride the
            # GPSIMD SWDGE queue.
            hT_src = hiddenT.rearrange("(ko p) s -> p ko s", p=128)
            w_src = w_qkv.rearrange("(ko p) n -> p ko n", p=128)
            for ko in range(KO):
                q = nc.sync if ko % 2 == 0 else nc.scalar
                q.dma_start(hT[:, ko, :], hT_src[:, ko, :])
                nc.gpsimd.dma_start(w_sb[:, ko, :], w_src[:, ko, :])
                if ko == 0:
                    nc.sync.dma_start(bv_sb, bv_rep[:, :])
                    nc.gpsimd.dma_start(dm_sb, diag_mask[:, :])
                    nc.gpsimd.dma_start(id_sb, ident[:, :])
                    nc.gpsimd.dma_start(lm_sb, logmult[:, :, :])
                    nc.gpsimd.dma_start(pw_sb, projw[:, :, :])

            # ========== qk projection round 0 (heads 0/1: nt 0 and 2) =====
            # ko (contraction) outer with 4 resident PSUM groups per pass;
            # two passes so the first pass's banks drain (DVE) while the
            # second computes, letting the attention pools start without
            # waiting for the full round to drain.
            with tc.tile_pool(name="p1ps", bufs=8, space="PSUM") as p1ps:
                for half in range(2):
                    ps4 = [p1ps.tile([128, 512], F32, tag="g",
                                     name=f"q{half}{i}")
                           for i in range(4)]
                    for ko in range(KO):
                        for i in range(4):
                            g = 4 * half + i
                            nt, sc = (0, 2)[g // 4], g % 4
                            nc.tensor.matmul(
                                ps4[i],
                                w_sb[:, ko, 128 * nt:128 * nt + 128],
                                hT[:, ko, 512 * sc:512 * sc + 512],
                                start=(ko == 0), stop=(ko == KO - 1),
                            )
                    for i in range(4):
                        g = 4 * half + i
                        nt, sc = (0, 2)[g // 4], g % 4
                        nc.vector.tensor_scalar_add(
                            qk_sb[:, nt, 512 * sc:512 * sc + 512], ps4[i],
                            bqk_sb[:, nt:nt + 1],
                        )

            # ================= attention + everything else =================
            # pool order fixes which freed qk-round banks each tag aliases:
            # ppr first so the w0 v-rounds can start as soon as the first
            # qk pass drains.
            with tc.tile_pool(name="ppr", bufs=2, space="PSUM") as ppr, \
                 tc.tile_pool(name="psc", bufs=2, space="PSUM") as psc, \
                 tc.tile_pool(name="pav", bufs=2, space="PSUM") as pav:

                stg_by = {}

                def emit_scores(lh, t):
                    """score pieces + exp into at bufs, then diag mask."""
                    bp = 64 * (lh % 2)
                    qn, kn = lh // 2, 2 + lh // 2
                    at_sb = at_by[lh]
                    gs = 128 * t
                    while gs < S:
                        w = min(1024, S - gs)
                        sc = psc.tile([128, 1024], F32, tag="sc")
                        o = 0
                        while o < w:
                            n = min(512, w - o)
                            nc.tensor.matmul(
                                sc[:, o:o + n],
                                qk_sb[bp:bp + 64, kn, 128 * t:128 * t + 128],
                                qk_sb[bp:bp + 64, qn, gs + o:gs + o + n],
                                start=True, stop=True,
                            )
                            o += n
                        nc.scalar.activation(
                            at_sb[:, OFF[t] + gs - 128 * t:
                                  OFF[t] + gs - 128 * t + w],
                            sc[:, :w],
                            mybir.ActivationFunctionType.Exp,
                            bias=lm_sb[:, lh, t:t + 1], scale=0.125,
                        )
                        gs += w
                    # causal 0/1 mask on the diagonal block (GPSIMD,
                    # all-SBUF, never gates ScalarE).
                    nc.gpsimd.tensor_mul(
                        out=at_by[lh][:, OFF[t]:OFF[t] + 128],
                        in0=at_by[lh][:, OFF[t]:OFF[t] + 128],
                        in1=dm_sb,
                    )

                def emit_av(lh, t):
                    """flipped AV for q-tile t + per-partition normalize."""
                    at_sb = at_by[lh]
                    av = pav.tile([128, 65], F32, tag="av", name=f"av{lh}{t}")
                    for u in range(t + 1):
                        nc.tensor.matmul(
                            av,
                            at_sb[:, OFF[u] + 128 * (t - u):
                                  OFF[u] + 128 * (t - u) + 128],
                            v_sb[:, u, lh, :],
                            start=(u == 0), stop=(u == t),
                        )
                    rec = recp.tile([128, 1], F32, tag="rec")
                    nc.vector.reciprocal(rec, av[:, 64:65])
                    nc.vector.tensor_scalar_mul(
                        stg_by[lh // 2][:, t, lh % 2, :], av[:, 0:64], rec,
                    )

                def emit_transpose(j, st):
                    """O' [q,(e,d)] block -> ao2 [(e,d), q] via PE."""
                    tp = pav.tile([128, 128], BF, tag="av", name=f"tp{j}{st}")
                    nc.tensor.matmul(
                        tp, stg_by[j][:, st, :, :], id_sb,
                        start=True, stop=True, is_transpose=True,
                    )
                    nc.vector.tensor_copy(
                        ao2[:, j, 128 * st:128 * st + 128], tp,
                    )

                def emit_cproj(st):
                    for ec in range(2):
                        pr = ppr.tile([128, 512], F32, tag="pr")
                        for jj in range(2):
                            nc.tensor.matmul(
                                pr,
                                ao2[:, jj, 128 * st:128 * st + 128],
                                pw_sb[:, jj, 512 * ec:512 * ec + 512],
                                start=(jj == 0), stop=(jj == 1),
                            )
                        o_sb = outp.tile([128, 512], BF, tag="osb")
                        nc.vector.tensor_copy(o_sb, pr)
                        nc.sync.dma_start(
                            out[128 * st:128 * st + 128,
                                512 * ec:512 * ec + 512],
                            o_sb,
                        )

                def emit_vround(st):
                    """v natural: out[s-tile, (h,d)] = hidden @ wv."""
                    ps = ppr.tile([128, 512], F32, tag="pr", name=f"v{st}")
                    for ko in range(KO):
                        nc.tensor.matmul(
                            ps[:, 0:256],
                            hT[:, ko, 128 * st:128 * st + 128],
                            w_sb[:, ko, 512:768],
                            start=(ko == 0), stop=(ko == KO - 1),
                        )
                    nc.vector.tensor_add(
                        out=v_sb[:, st, :, 0:64],
                        in0=ps[:, 0:256].rearrange("p (h d) -> p h d", d=64),
                        in1=bv_sb.rearrange("p (h d) -> p h d", d=64),
                    )

                def emit_qkr1(g):
                    """deferred qk projection for heads 2/3 (nt 1 and 3)."""
                    nt, sc4 = (1, 3)[g // 4], g % 4
                    ps = ppr.tile([128, 512], F32, tag="pr", name=f"qr{g}")
                    for ko in range(KO):
                        nc.tensor.matmul(
                            ps,
                            w_sb[:, ko, 128 * nt:128 * nt + 128],
                            hT[:, ko, 512 * sc4:512 * sc4 + 512],
                            start=(ko == 0), stop=(ko == KO - 1),
                        )
                    nc.vector.tensor_scalar_add(
                        qk_sb[:, nt, 512 * sc4:512 * sc4 + 512], ps,
                        bqk_sb[:, nt:nt + 1],
                    )

                at_by = {}

                def new_at(lh):
                    at_by[lh] = atp.tile([128, ATW], BF, tag="at",
                                         name=f"at{lh}")
                    if lh // 2 not in stg_by:
                        stg_by[lh // 2] = stgp.tile([128, NT, 2, 64], BF,
                                                    tag="stg",
                                                    name=f"stg{lh // 2}")

                # ---- w0: head 0, v-projection rounds as PE filler ----
                new_at(0)
                for t in range(NT):
                    emit_vround(t)
                    emit_scores(0, t)
                    if t >= 1:
                        emit_av(0, t - 1)
                emit_av(0, NT - 1)

                # ---- w1: head 1 + all-nt1 and first-nt3 qk groups ----
                new_at(1)
                for t in range(NT):
                    if t < 5:
                        emit_qkr1(t)
                    emit_scores(1, t)
                    if t >= 1:
                        emit_av(1, t - 1)
                emit_av(1, NT - 1)

                # ---- w2: head 3; early-round PE fillers are the rest of
                # the nt3 qk groups and the pair-0 transposes, late rounds
                # pull head-2 score tiles 0..4 ahead ----
                new_at(3)
                for t in range(NT):
                    if 1 <= t <= 3:
                        emit_qkr1(4 + t)
                    emit_scores(3, t)
                    if 4 <= t <= 11:
                        emit_transpose(0, 2 * (t - 4))
                        emit_transpose(0, 2 * (t - 4) + 1)
                    if t >= 1:
                        emit_av(3, t - 1)
                    if t >= 11:
                        if t == 11:
                            new_at(2)
                        emit_scores(2, t - 11)
                emit_av(3, NT - 1)

                # ---- w3: head 2 rest + pair-1 transposes + c_proj ----
                for t in range(NT):
                    if t >= 5:
                        emit_scores(2, t)
                    if t >= 1:
                        emit_av(2, t - 1)
                    if t >= 2:
                        emit_transpose(1, t - 2)
                    if t >= 3:
                        emit_cproj(t - 3)
                emit_av(2, NT - 1)
                emit_transpose(1, NT - 2)
                emit_transpose(1, NT - 1)
                for st in (NT - 3, NT - 2, NT - 1):
                    emit_cproj(st)
    return nc


_NC = None


def _get_nc():
    global _NC
    if _NC is None:
        _NC = build_program()
    return _NC


# ---------------------------------------------------------------- host prep

def make_in_maps(hidden_states, c_attn_w, c_attn_b, c_proj_w):
    first_end = S // 3
    second_end = 2 * S // 3
    pos = np.arange(S)
    regions = [pos < first_end,
               (pos >= first_end) & (pos < second_end),
               pos >= second_end]
    mult = np.ones((H, S), dtype=np.float64)
    for h, r in HEAD_REGION.items():
        mult[h] = 1.0 + (FOCUS - 1.0) * regions[r].astype(np.float64)
    logm = np.log(mult).astype(np.float32)  # [H, S]

    p = np.arange(128)[:, None]
    j = np.arange(128)[None, :]
    diag = (j >= p).astype(np.float32)  # 0/1 keep-mask, applied post-exp
    iden = (j == p).astype(np.float32)

    bf = ml_dtypes.bfloat16
    in_maps = []
    for c in range(NCORES):
        b, g = divmod(c, GROUPS)
        h0 = HPC * g
        cs = slice(256 * g, 256 * g + 256)
        w_qkv = np.concatenate(
            [c_attn_w[:, cs], c_attn_w[:, 1024:2048][:, cs],
             c_attn_w[:, 2048:3072][:, cs]], axis=1,
        ).astype(bf)
        bqk_h = np.concatenate(
            [c_attn_b[cs], c_attn_b[1024:2048][cs]]
        ).reshape(4, 128).T.copy().astype(np.float32)
        bv = np.broadcast_to(
            c_attn_b[2048:3072][cs], (128, 256)
        ).astype(np.float32).copy()
        # pw2[p, j, e]: head pair j=(2j, 2j+1); p<64 -> head 2j row p,
        # p>=64 -> head 2j+1 row p-64  (matches ao2 partition interleave)
        pw = c_proj_w[64 * h0:64 * h0 + 256, :].reshape(2, 128, D)
        pw = np.ascontiguousarray(pw.transpose(1, 0, 2)).astype(bf)
        lm = logm[h0:h0 + HPC].reshape(HPC, S // 128, 128)
        lm = np.ascontiguousarray(lm.transpose(2, 0, 1)).astype(np.float32)
        in_maps.append({
            "hiddenT": np.ascontiguousarray(hidden_states[b].T).astype(bf),
            "w_qkv": w_qkv,
            "bqk": bqk_h,
            "bv_rep": bv,
            "projw": pw,
            "diag_mask": diag.astype(bf),
            "ident": iden.astype(bf),
            "logmult": lm,
        })
    return in_maps


def run_cores(in_maps, trace=False, **kw):
    from concourse.bass_utils import run_bass_kernel_spmd
    nc = _get_nc()
    return run_bass_kernel_spmd(nc, in_maps, core_ids=list(range(NCORES)),
                                trace=trace, **kw)


def kernel(hidden_states, c_attn_w, c_attn_b, c_proj_w, c_proj_b):
    hidden_states = np.asarray(hidden_states, dtype=np.float32)
    c_attn_w = np.asarray(c_attn_w, dtype=np.float32)
    c_attn_b = np.asarray(c_attn_b, dtype=np.float32)
    c_proj_w = np.asarray(c_proj_w, dtype=np.float32)
    c_proj_b = np.asarray(c_proj_b, dtype=np.float32)

    in_maps = make_in_maps(hidden_states, c_attn_w, c_attn_b, c_proj_w)
    res = run_cores(in_maps)
    out = np.zeros((B, S, D), dtype=np.float32)
    for c in range(NCORES):
        out[c // GROUPS] += np.asarray(res.results[c]["out"],
                                       dtype=np.float32)
    out += c_proj_b[None, None, :]
    return out


# revision 9
# speedup vs baseline: 1.2191x; 1.0553x over previous
"""DivergentAttention Trainium2 kernel (8 NeuronCores, Bass/Tile).

Problem: GPT-2 style causal self-attention (B=2, S=2048, D=1024, H=16,
hd=64) where heads 0/1/2 re-weight their attention toward a token region
(first/middle/last third of the sequence) with factor 1.6 and renormalize.

Key identity: softmax(s)*m / sum(softmax(s)*m) == softmax(s + log m), so the
per-head region reweight folds into an additive per-(head, key-position)
bias on the scores -- no second normalization pass needed.  Scores are small
(|s|<~5) so the max-subtraction pass is skipped entirely.

Sharding: tensor-parallel over (batch, head-group): core c handles batch
c//4 and heads [4*(c%4), 4*(c%4)+4).  Each core computes the QKV projection
for its 4 heads, full causal attention, and its partial c_proj; the host
sums the 8 partials (fp32) and adds c_proj_b.

All matmul inputs are bf16 (error budget allows it; bf16 runs the PE at one
row per output column for every width, unlike fp32r which needs N>=256).

Attention structure per head (the big change vs the v1 kernel):
  - scoresT [sk-tile 128, q] = kT.T @ qT as before, exp'd (ScalarE, scale
    1/8, bias log mult) into a per-head PERSISTENT attn buffer at_sb
    [128, 17408] bf16 holding the whole causal triangle.
  - AV is FLIPPED: out[q-tile 128, 65] = attnT(stationary).T @ v_aug --
    65 output columns per (q-tile, sk-tile) pair instead of 512-wide
    rows, halving AV PE time.  v_aug column 64 is ones, so column 64 of
    the output is the softmax denominator per q ROW -- normalization
    becomes a per-partition scalar multiply (DVE reciprocal + mult), no
    partition-broadcast DMA bounce.
  - The normalized per-head output O' [q, 64] bf16 is transposed back to
    [64*(h%2)+d, q] for c_proj with a PE transpose (identity moving
    operand), 128 rows per q-tile.
  - c_proj: two K=128 matmuls per output tile against pw2 [128, 2, D].

Window schedule (PE-balance: exp on ScalarE is the per-head pacing limit,
so PE filler work is spread into the exp-bound windows):
  w0: head 0 + one v-projection tile per round (pr-tag PSUM).
  w1: head 1 + the deferred qk projection round for heads 2/3 (8 groups,
      rounds 0..7) + pair-0 transposes.
  w2: head 3 + head-2 score tiles 0..4 pulled ahead (rounds 11..15).
  w3: head 2 (scores 5..15) + pair-1 transposes + c_proj, software-
      pipelined (transpose at round t-2, c_proj at round t-3) so the PE
      never waits on the DVE/ScalarE drains.
"""

import numpy as np
import ml_dtypes

import concourse.bass as bass
import concourse.tile as tile
from concourse import mybir
from concourse import bass_utils, bass2jax

# ---------------------------------------------------------------- constants
B, S, D, H, HD = 2, 2048, 1024, 16, 64
NCORES = 8
HPC = 4              # heads per core
GROUPS = 4           # head groups
FOCUS = 1.6
HEAD_REGION = {0: 0, 1: 1, 2: 2}
NT = S // 128        # 16 sk/q tiles
KO = D // 128        # 8 contraction chunks
BF = mybir.dt.bfloat16
F32 = mybir.dt.float32

# column offset of tile t's rows inside the per-head attn buffer
OFF = [0] * (NT + 1)
for _t in range(NT):
    OFF[_t + 1] = OFF[_t] + (S - 128 * _t)
ATW = OFF[NT]        # 17408

# ------------------------------------------------- walrus multi-wait fixup
# This container's walrus accepts only ONE sync-wait per TPB instruction,
# but Tile attaches one wait per dependency proc.  Rewrite the BIR JSON just
# before walrus: hoist all-but-one wait of a multi-wait instruction onto
# standalone same-engine NoOps inserted immediately before it (same-engine
# program order is preserved, so semantics are unchanged).
try:
    import orjson as _json
except ImportError:  # pragma: no cover
    import json as _json

_orig_compile_bir_kernel = bass_utils.compile_bir_kernel
_wfix_counter = [0]


def _fix_bir(bir_json):
    d = _json.loads(bir_json)
    changed = False
    for fn in d.get("functions", []):
        for blk in fn.get("blocks", []):
            out = []
            for inst in blk.get("instructions", []):
                si = inst.get("sync_info")
                if si:
                    waits = si.get("on_wait") or []
                    if len(waits) > 1:
                        changed = True
                        for w in waits[:-1]:
                            _wfix_counter[0] += 1
                            nop = {
                                "engine": inst["engine"],
                                "ins": [],
                                "name": f"I-wfix-{_wfix_counter[0]}",
                                "opcode": "NoOp",
                                "outs": [],
                                "sync_info": {"on_update": [], "on_wait": [w]},
                            }
                            if "debug" in inst:
                                nop["debug"] = inst["debug"]
                            out.append(nop)
                        si["on_wait"] = waits[-1:]
                out.append(inst)
            blk["instructions"] = out
    return _json.dumps(d) if changed else bir_json


def _patched_compile_bir_kernel(bir_json, tmpdir, neff_name="file.neff"):
    return _orig_compile_bir_kernel(_fix_bir(bir_json), tmpdir, neff_name=neff_name)


def _install_waitfix():
    bass_utils.compile_bir_kernel = _patched_compile_bir_kernel
    bass2jax.compile_bir_kernel = _patched_compile_bir_kernel


_install_waitfix()

# ---------------------------------------------------------------- program


def build_program():
    """One SPMD Bass program; per-core differences come in via inputs."""
    nc = bass.Bass()

    hiddenT = nc.dram_tensor("hiddenT", [D, S], BF, kind="ExternalInput")
    w_qkv = nc.dram_tensor("w_qkv", [D, 768], BF, kind="ExternalInput")
    bqk = nc.dram_tensor("bqk", [128, 4], F32, kind="ExternalInput")
    bv_rep = nc.dram_tensor("bv_rep", [128, 256], F32, kind="ExternalInput")
    projw = nc.dram_tensor("projw", [128, 2, D], BF, kind="ExternalInput")
    diag_mask = nc.dram_tensor("diag_mask", [128, 128], BF, kind="ExternalInput")
    ident = nc.dram_tensor("ident", [128, 128], BF, kind="ExternalInput")
    logmult = nc.dram_tensor("logmult", [128, HPC, NT], F32, kind="ExternalInput")
    out = nc.dram_tensor("out", [S, D], BF, kind="ExternalOutput")

    with tile.TileContext(nc) as tc:
        with tc.tile_pool(name="persist", bufs=1) as persist, \
             tc.tile_pool(name="atp", bufs=2) as atp, \
             tc.tile_pool(name="stgp", bufs=2) as stgp, \
             tc.tile_pool(name="recp", bufs=4) as recp, \
             tc.tile_pool(name="outp", bufs=4) as outp:

            # ---- persistent SBUF ----
            hT = persist.tile([128, KO, S], BF)           # 4 MB
            w_sb = persist.tile([128, KO, 768], BF)       # 1.5 MB
            qk_sb = persist.tile([128, 4, S], BF)         # 2 MB
            v_sb = persist.tile([128, NT, HPC, 65], BF)   # ~1 MB
            ao2 = persist.tile([128, 2, S], BF)           # attn_outT, 1 MB
            bqk_sb = persist.tile([128, 4], F32)
            bv_sb = persist.tile([128, 256], F32)
            pw_sb = persist.tile([128, 2, D], BF)         # 0.5 MB
            dm_sb = persist.tile([128, 128], BF)
            id_sb = persist.tile([128, 128], BF)
            lm_sb = persist.tile([128, HPC, NT], F32)

            nc.sync.dma_start(bqk_sb, bqk[:, :])
            nc.vector.memset(v_sb[:, :, :, 64:65], 1.0)

            # input loads: hT alternates the SP and ACT HWDGE queues (ACT
            # is idle until the first exp), w + small tensors ride the
            # GPSIMD SWDGE queue.
            hT_src = hiddenT.rearrange("(ko p) s -> p ko s", p=128)
            w_src = w_qkv.rearrange("(ko p) n -> p ko n", p=128)
            # first w slice + first hT chunk in small pieces so the first
            # matmul can fire as early as possible
            nc.sync.dma_start(w_sb[:, 0, 0:128], w_src[:, 0, 0:128])
            for pc in range(4):
                nc.sync.dma_start(hT[:, 0, 512 * pc:512 * pc + 512],
                                  hT_src[:, 0, 512 * pc:512 * pc + 512])
            nc.gpsimd.dma_start(w_sb[:, 0, 128:768], w_src[:, 0, 128:768])
            for ko in range(1, KO):
                q = nc.sync if ko % 2 == 0 else nc.scalar
                q.dma_start(hT[:, ko, :], hT_src[:, ko, :])
                nc.gpsimd.dma_start(w_sb[:, ko, :], w_src[:, ko, :])
                if ko == 1:
                    nc.sync.dma_start(bv_sb, bv_rep[:, :])
                    nc.gpsimd.dma_start(dm_sb, diag_mask[:, :])
                    nc.gpsimd.dma_start(id_sb, ident[:, :])
                    nc.gpsimd.dma_start(lm_sb, logmult[:, :, :])
                    nc.gpsimd.dma_start(pw_sb, projw[:, :, :])

            # ========== qk projection round 0 (heads 0/1: nt 0 and 2) =====
            # ko (contraction) outer with 4 resident PSUM groups per pass;
            # two passes so the first pass's banks drain (DVE) while the
            # second computes, letting the attention pools start without
            # waiting for the full round to drain.
            with tc.tile_pool(name="p1ps", bufs=8, space="PSUM") as p1ps:
                # pass 1 (nt 0): ko outer, paced by the hT loads
                ps4 = [p1ps.tile([128, 512], F32, tag="g", name=f"q0{i}")
                       for i in range(4)]
                for ko in range(KO):
                    for i in range(4):
                        nc.tensor.matmul(
                            ps4[i],
                            w_sb[:, ko, 0:128],
                            hT[:, ko, 512 * i:512 * i + 512],
                            start=(ko == 0), stop=(ko == KO - 1),
                        )
                for i in range(4):
                    nc.vector.tensor_scalar_add(
                        qk_sb[:, 0, 512 * i:512 * i + 512], ps4[i],
                        bqk_sb[:, 0:1],
                    )
                # pass 2 (nt 2): everything resident now -> ko inner with a
                # drain right after each group, so the banks free quickly
                for i in range(4):
                    ps = p1ps.tile([128, 512], F32, tag="g", name=f"q1{i}")
                    for ko in range(KO):
                        nc.tensor.matmul(
                            ps,
                            w_sb[:, ko, 256:384],
                            hT[:, ko, 512 * i:512 * i + 512],
                            start=(ko == 0), stop=(ko == KO - 1),
                        )
                    nc.vector.tensor_scalar_add(
                        qk_sb[:, 2, 512 * i:512 * i + 512], ps,
                        bqk_sb[:, 2:3],
                    )

            # ================= attention + everything else =================
            # pool order fixes which freed qk-round banks each tag aliases:
            # ppr first so the w0 v-rounds can start as soon as the first
            # qk pass drains.
            with tc.tile_pool(name="ppr", bufs=2, space="PSUM") as ppr, \
                 tc.tile_pool(name="psc", bufs=2, space="PSUM") as psc, \
                 tc.tile_pool(name="pav", bufs=2, space="PSUM") as pav:

                stg_by = {}

                def emit_scores(lh, t):
                    """score pieces + exp into at bufs, then diag mask."""
                    bp = 64 * (lh % 2)
                    qn, kn = lh // 2, 2 + lh // 2
                    at_sb = at_by[lh]
                    gs = 128 * t
                    while gs < S:
                        w = min(1024, S - gs)
                        sc = psc.tile([128, 1024], F32, tag="sc")
                        o = 0
                        while o < w:
                            n = min(512, w - o)
                            nc.tensor.matmul(
                                sc[:, o:o + n],
                                qk_sb[bp:bp + 64, kn, 128 * t:128 * t + 128],
                                qk_sb[bp:bp + 64, qn, gs + o:gs + o + n],
                                start=True, stop=True,
                            )
                            o += n
                        nc.scalar.activation(
                            at_sb[:, OFF[t] + gs - 128 * t:
                                  OFF[t] + gs - 128 * t + w],
                            sc[:, :w],
                            mybir.ActivationFunctionType.Exp,
                            bias=lm_sb[:, lh, t:t + 1], scale=0.125,
                        )
                        gs += w
                    # causal 0/1 mask on the diagonal block (GPSIMD,
                    # all-SBUF, never gates ScalarE).
                    nc.gpsimd.tensor_mul(
                        out=at_by[lh][:, OFF[t]:OFF[t] + 128],
                        in0=at_by[lh][:, OFF[t]:OFF[t] + 128],
                        in1=dm_sb,
                    )

                def emit_av(lh, t):
                    """flipped AV for q-tile t + per-partition normalize."""
                    at_sb = at_by[lh]
                    av = pav.tile([128, 65], F32, tag="av", name=f"av{lh}{t}")
                    for u in range(t + 1):
                        nc.tensor.matmul(
                            av,
                            at_sb[:, OFF[u] + 128 * (t - u):
                                  OFF[u] + 128 * (t - u) + 128],
                            v_sb[:, u, lh, :],
                            start=(u == 0), stop=(u == t),
                        )
                    rec = recp.tile([128, 1], F32, tag="rec")
                    nc.vector.reciprocal(rec, av[:, 64:65])
                    nc.vector.tensor_scalar_mul(
                        stg_by[lh // 2][:, t, lh % 2, :], av[:, 0:64], rec,
                    )

                def emit_transpose(j, st):
                    """O' [q,(e,d)] block -> ao2 [(e,d), q] via PE."""
                    tp = pav.tile([128, 128], BF, tag="av", name=f"tp{j}{st}")
                    nc.tensor.matmul(
                        tp, stg_by[j][:, st, :, :], id_sb,
                        start=True, stop=True, is_transpose=True,
                    )
                    nc.vector.tensor_copy(
                        ao2[:, j, 128 * st:128 * st + 128], tp,
                    )

                def emit_cproj(st):
                    for ec in range(2):
                        pr = ppr.tile([128, 512], F32, tag="pr")
                        for jj in range(2):
                            nc.tensor.matmul(
                                pr,
                                ao2[:, jj, 128 * st:128 * st + 128],
                                pw_sb[:, jj, 512 * ec:512 * ec + 512],
                                start=(jj == 0), stop=(jj == 1),
                            )
                        o_sb = outp.tile([128, 512], BF, tag="osb")
                        nc.vector.tensor_copy(o_sb, pr)
                        nc.sync.dma_start(
                            out[128 * st:128 * st + 128,
                                512 * ec:512 * ec + 512],
                            o_sb,
                        )

                def emit_vround(st):
                    """v natural: out[s-tile, (h,d)] = hidden @ wv."""
                    ps = ppr.tile([128, 512], F32, tag="pr", name=f"v{st}")
                    for ko in range(KO):
                        nc.tensor.matmul(
                            ps[:, 0:256],
                            hT[:, ko, 128 * st:128 * st + 128],
                            w_sb[:, ko, 512:768],
                            start=(ko == 0), stop=(ko == KO - 1),
                        )
                    nc.vector.tensor_add(
                        out=v_sb[:, st, :, 0:64],
                        in0=ps[:, 0:256].rearrange("p (h d) -> p h d", d=64),
                        in1=bv_sb.rearrange("p (h d) -> p h d", d=64),
                    )

                def emit_qkr1(g):
                    """deferred qk projection for heads 2/3 (nt 1 and 3)."""
                    nt, sc4 = (1, 3)[g // 4], g % 4
                    ps = ppr.tile([128, 512], F32, tag="pr", name=f"qr{g}")
                    for ko in range(KO):
                        nc.tensor.matmul(
                            ps,
                            w_sb[:, ko, 128 * nt:128 * nt + 128],
                            hT[:, ko, 512 * sc4:512 * sc4 + 512],
                            start=(ko == 0), stop=(ko == KO - 1),
                        )
                    nc.vector.tensor_scalar_add(
                        qk_sb[:, nt, 512 * sc4:512 * sc4 + 512], ps,
                        bqk_sb[:, nt:nt + 1],
                    )

                at_by = {}

                def new_at(lh):
                    at_by[lh] = atp.tile([128, ATW], BF, tag="at",
                                         name=f"at{lh}")
                    if lh // 2 not in stg_by:
                        stg_by[lh // 2] = stgp.tile([128, NT, 2, 64], BF,
                                                    tag="stg",
                                                    name=f"stg{lh // 2}")

                # exp -> AV pipeline depth: AV for q-tile t runs LAG score
                # rounds later so transiently exp-bound rounds never stall
                # the PE on a missing exp.
                LAG = 2

                # ---- w0: head 0, v-projection rounds as PE filler ----
                new_at(0)
                for t in range(NT):
                    emit_vround(t)
                    emit_scores(0, t)
                    if t >= LAG:
                        emit_av(0, t - LAG)
                for u in range(NT - LAG, NT):
                    emit_av(0, u)

                # ---- w1: head 1 + all-nt1 and first-nt3 qk groups ----
                new_at(1)
                for t in range(NT):
                    if t < 5:
                        emit_qkr1(t)
                    emit_scores(1, t)
                    if t >= LAG:
                        emit_av(1, t - LAG)
                for u in range(NT - LAG, NT):
                    emit_av(1, u)

                # ---- w2: head 3; early-round PE fillers are the rest of
                # the nt3 qk groups and the pair-0 transposes ----
                new_at(3)
                for t in range(NT):
                    if 1 <= t <= 3:
                        emit_qkr1(4 + t)
                    emit_scores(3, t)
                    if 4 <= t <= 11:
                        emit_transpose(0, 2 * (t - 4))
                        emit_transpose(0, 2 * (t - 4) + 1)
                    if t >= LAG:
                        emit_av(3, t - LAG)
                for u in range(NT - LAG, NT):
                    emit_av(3, u)

                # ---- w3: head 2 + pair-1 transposes + c_proj, software-
                # pipelined behind the AV/norm drains ----
                new_at(2)
                for t in range(NT):
                    emit_scores(2, t)
                    if t >= LAG:
                        emit_av(2, t - LAG)
                    if t >= LAG + 1:
                        emit_transpose(1, t - LAG - 1)
                    if t >= LAG + 2:
                        emit_cproj(t - LAG - 2)
                for u in range(NT - LAG, NT):
                    emit_av(2, u)
                for st in range(NT - LAG - 1, NT):
                    emit_transpose(1, st)
                for st in range(NT - LAG - 2, NT):
                    emit_cproj(st)
    return nc


_NC = None


def _get_nc():
    global _NC
    if _NC is None:
        _NC = build_program()
    return _NC


# ---------------------------------------------------------------- host prep

def make_in_maps(hidden_states, c_attn_w, c_attn_b, c_proj_w):
    first_end = S // 3
    second_end = 2 * S // 3
    pos = np.arange(S)
    regions = [pos < first_end,
               (pos >= first_end) & (pos < second_end),
               pos >= second_end]
    mult = np.ones((H, S), dtype=np.float64)
    for h, r in HEAD_REGION.items():
        mult[h] = 1.0 + (FOCUS - 1.0) * regions[r].astype(np.float64)
    logm = np.log(mult).astype(np.float32)  # [H, S]

    p = np.arange(128)[:, None]
    j = np.arange(128)[None, :]
    diag = (j >= p).astype(np.float32)  # 0/1 keep-mask, applied post-exp
    iden = (j == p).astype(np.float32)

    bf = ml_dtypes.bfloat16
    in_maps = []
    for c in range(NCORES):
        b, g = divmod(c, GROUPS)
        h0 = HPC * g
        cs = slice(256 * g, 256 * g + 256)
        w_qkv = np.concatenate(
            [c_attn_w[:, cs], c_attn_w[:, 1024:2048][:, cs],
             c_attn_w[:, 2048:3072][:, cs]], axis=1,
        ).astype(bf)
        bqk_h = np.concatenate(
            [c_attn_b[cs], c_attn_b[1024:2048][cs]]
        ).reshape(4, 128).T.copy().astype(np.float32)
        bv = np.broadcast_to(
            c_attn_b[2048:3072][cs], (128, 256)
        ).astype(np.float32).copy()
        # pw2[p, j, e]: head pair j=(2j, 2j+1); p<64 -> head 2j row p,
        # p>=64 -> head 2j+1 row p-64  (matches ao2 partition interleave)
        pw = c_proj_w[64 * h0:64 * h0 + 256, :].reshape(2, 128, D)
        pw = np.ascontiguousarray(pw.transpose(1, 0, 2)).astype(bf)
        lm = logm[h0:h0 + HPC].reshape(HPC, S // 128, 128)
        lm = np.ascontiguousarray(lm.transpose(2, 0, 1)).astype(np.float32)
        in_maps.append({
            "hiddenT": np.ascontiguousarray(hidden_states[b].T).astype(bf),
            "w_qkv": w_qkv,
            "bqk": bqk_h,
            "bv_rep": bv,
            "projw": pw,
            "diag_mask": diag.astype(bf),
            "ident": iden.astype(bf),
            "logmult": lm,
        })
    return in_maps


def run_cores(in_maps, trace=False, **kw):
    from concourse.bass_utils import run_bass_kernel_spmd
    nc = _get_nc()
    return run_bass_kernel_spmd(nc, in_maps, core_ids=list(range(NCORES)),
                                trace=trace, **kw)


def kernel(hidden_states, c_attn_w, c_attn_b, c_proj_w, c_proj_b):
    hidden_states = np.asarray(hidden_states, dtype=np.float32)
    c_attn_w = np.asarray(c_attn_w, dtype=np.float32)
    c_attn_b = np.asarray(c_attn_b, dtype=np.float32)
    c_proj_w = np.asarray(c_proj_w, dtype=np.float32)
    c_proj_b = np.asarray(c_proj_b, dtype=np.float32)

    in_maps = make_in_maps(hidden_states, c_attn_w, c_attn_b, c_proj_w)
    res = run_cores(in_maps)
    out = np.zeros((B, S, D), dtype=np.float32)
    for c in range(NCORES):
        out[c // GROUPS] += np.asarray(res.results[c]["out"],
                                       dtype=np.float32)
    out += c_proj_b[None, None, :]
    return out


# revision 13
# speedup vs baseline: 1.2229x; 1.0031x over previous
"""DivergentAttention Trainium2 kernel (8 NeuronCores, Bass/Tile).

Problem: GPT-2 style causal self-attention (B=2, S=2048, D=1024, H=16,
hd=64) where heads 0/1/2 re-weight their attention toward a token region
(first/middle/last third of the sequence) with factor 1.6 and renormalize.

Key identity: softmax(s)*m / sum(softmax(s)*m) == softmax(s + log m), so the
per-head region reweight folds into an additive per-(head, key-position)
bias on the scores -- no second normalization pass needed.  Scores are small
(|s|<~5) so the max-subtraction pass is skipped entirely.

Sharding: tensor-parallel over (batch, head-group): core c handles batch
c//4 and heads [4*(c%4), 4*(c%4)+4).  Each core computes the QKV projection
for its 4 heads, full causal attention, and its partial c_proj; the host
sums the 8 partials (fp32) and adds c_proj_b.

All matmul inputs are bf16 (error budget allows it; bf16 runs the PE at one
row per output column for every width, unlike fp32r which needs N>=256).

Attention structure per head (the big change vs the v1 kernel):
  - scoresT [sk-tile 128, q] = kT.T @ qT as before, exp'd (ScalarE, scale
    1/8, bias log mult) into a per-head PERSISTENT attn buffer at_sb
    [128, 17408] bf16 holding the whole causal triangle.
  - AV is FLIPPED: out[q-tile 128, 65] = attnT(stationary).T @ v_aug --
    65 output columns per (q-tile, sk-tile) pair instead of 512-wide
    rows, halving AV PE time.  v_aug column 64 is ones, so column 64 of
    the output is the softmax denominator per q ROW -- normalization
    becomes a per-partition scalar multiply (DVE reciprocal + mult), no
    partition-broadcast DMA bounce.
  - The normalized per-head output O' [q, 64] bf16 is transposed back to
    [64*(h%2)+d, q] for c_proj with a PE transpose (identity moving
    operand), 128 rows per q-tile.
  - c_proj: two K=128 matmuls per output tile against pw2 [128, 2, D].

Window schedule (PE-balance: exp on ScalarE is the per-head pacing limit,
so PE filler work is spread into the exp-bound windows):
  w0: head 0 + one v-projection tile per round (pr-tag PSUM).
  w1: head 1 + the deferred qk projection round for heads 2/3 (8 groups,
      rounds 0..7) + pair-0 transposes.
  w2: head 3 + head-2 score tiles 0..4 pulled ahead (rounds 11..15).
  w3: head 2 (scores 5..15) + pair-1 transposes + c_proj, software-
      pipelined (transpose at round t-2, c_proj at round t-3) so the PE
      never waits on the DVE/ScalarE drains.
"""

import numpy as np
import ml_dtypes

import concourse.bass as bass
import concourse.tile as tile
from concourse import mybir
from concourse import bass_utils, bass2jax

# ---------------------------------------------------------------- constants
B, S, D, H, HD = 2, 2048, 1024, 16, 64
NCORES = 8
HPC = 4              # heads per core
GROUPS = 4           # head groups
FOCUS = 1.6
HEAD_REGION = {0: 0, 1: 1, 2: 2}
NT = S // 128        # 16 sk/q tiles
KO = D // 128        # 8 contraction chunks
BF = mybir.dt.bfloat16
F32 = mybir.dt.float32

# column offset of tile t's rows inside the per-head attn buffer
OFF = [0] * (NT + 1)
for _t in range(NT):
    OFF[_t + 1] = OFF[_t] + (S - 128 * _t)
ATW = OFF[NT]        # 17408

# ------------------------------------------------- walrus multi-wait fixup
# This container's walrus accepts only ONE sync-wait per TPB instruction,
# but Tile attaches one wait per dependency proc.  Rewrite the BIR JSON just
# before walrus: hoist all-but-one wait of a multi-wait instruction onto
# standalone same-engine NoOps inserted immediately before it (same-engine
# program order is preserved, so semantics are unchanged).
try:
    import orjson as _json
except ImportError:  # pragma: no cover
    import json as _json

_orig_compile_bir_kernel = bass_utils.compile_bir_kernel
_wfix_counter = [0]


def _fix_bir(bir_json):
    d = _json.loads(bir_json)
    changed = False
    for fn in d.get("functions", []):
        for blk in fn.get("blocks", []):
            out = []
            for inst in blk.get("instructions", []):
                si = inst.get("sync_info")
                if si:
                    waits = si.get("on_wait") or []
                    if len(waits) > 1:
                        changed = True
                        for w in waits[:-1]:
                            _wfix_counter[0] += 1
                            nop = {
                                "engine": inst["engine"],
                                "ins": [],
                                "name": f"I-wfix-{_wfix_counter[0]}",
                                "opcode": "NoOp",
                                "outs": [],
                                "sync_info": {"on_update": [], "on_wait": [w]},
                            }
                            if "debug" in inst:
                                nop["debug"] = inst["debug"]
                            out.append(nop)
                        si["on_wait"] = waits[-1:]
                out.append(inst)
            blk["instructions"] = out
    return _json.dumps(d) if changed else bir_json


def _patched_compile_bir_kernel(bir_json, tmpdir, neff_name="file.neff"):
    return _orig_compile_bir_kernel(_fix_bir(bir_json), tmpdir, neff_name=neff_name)


def _install_waitfix():
    bass_utils.compile_bir_kernel = _patched_compile_bir_kernel
    bass2jax.compile_bir_kernel = _patched_compile_bir_kernel


_install_waitfix()

# ---------------------------------------------------------------- program


def build_program():
    """One SPMD Bass program; per-core differences come in via inputs."""
    nc = bass.Bass()

    hiddenT = nc.dram_tensor("hiddenT", [D, S], BF, kind="ExternalInput")
    w_qkv = nc.dram_tensor("w_qkv", [D, 768], BF, kind="ExternalInput")
    bqk = nc.dram_tensor("bqk", [128, 4], F32, kind="ExternalInput")
    bv_rep = nc.dram_tensor("bv_rep", [128, 256], F32, kind="ExternalInput")
    projw = nc.dram_tensor("projw", [128, 2, D], BF, kind="ExternalInput")
    diag_mask = nc.dram_tensor("diag_mask", [128, 128], BF, kind="ExternalInput")
    ident = nc.dram_tensor("ident", [128, 128], BF, kind="ExternalInput")
    logmult = nc.dram_tensor("logmult", [128, HPC, NT], F32, kind="ExternalInput")
    out = nc.dram_tensor("out", [S, D], BF, kind="ExternalOutput")

    with tile.TileContext(nc) as tc:
        with tc.tile_pool(name="persist", bufs=1) as persist, \
             tc.tile_pool(name="atp", bufs=2) as atp, \
             tc.tile_pool(name="stgp", bufs=2) as stgp, \
             tc.tile_pool(name="recp", bufs=4) as recp, \
             tc.tile_pool(name="outp", bufs=4) as outp:

            # ---- persistent SBUF ----
            hT = persist.tile([128, KO, S], BF)           # 4 MB
            w_sb = persist.tile([128, KO, 768], BF)       # 1.5 MB
            qk_sb = persist.tile([128, 4, S], BF)         # 2 MB
            v_sb = persist.tile([128, NT, HPC, 65], BF)   # ~1 MB
            ao2 = persist.tile([128, 2, S], BF)           # attn_outT, 1 MB
            bqk_sb = persist.tile([128, 4], F32)
            bv_sb = persist.tile([128, 256], F32)
            pw_sb = persist.tile([128, 2, D], BF)         # 0.5 MB
            dm_sb = persist.tile([128, 128], BF)
            id_sb = persist.tile([128, 128], BF)
            lm_sb = persist.tile([128, HPC, NT], F32)

            nc.sync.dma_start(bqk_sb, bqk[:, :])
            nc.vector.memset(v_sb[:, :, :, 64:65], 1.0)

            # input loads: hT alternates the SP and ACT HWDGE queues (ACT
            # is idle until the first exp), w + small tensors ride the
            # GPSIMD SWDGE queue.
            hT_src = hiddenT.rearrange("(ko p) s -> p ko s", p=128)
            w_src = w_qkv.rearrange("(ko p) n -> p ko n", p=128)
            # first w slice + first hT chunk in small pieces so the first
            # matmul can fire as early as possible
            nc.sync.dma_start(w_sb[:, 0, 0:128], w_src[:, 0, 0:128])
            for pc in range(4):
                nc.sync.dma_start(hT[:, 0, 512 * pc:512 * pc + 512],
                                  hT_src[:, 0, 512 * pc:512 * pc + 512])
            nc.gpsimd.dma_start(w_sb[:, 0, 128:768], w_src[:, 0, 128:768])
            for ko in range(1, KO):
                q = nc.sync if ko % 2 == 0 else nc.scalar
                q.dma_start(hT[:, ko, :], hT_src[:, ko, :])
                nc.gpsimd.dma_start(w_sb[:, ko, :], w_src[:, ko, :])
                if ko == 1:
                    nc.sync.dma_start(bv_sb, bv_rep[:, :])
                    nc.gpsimd.dma_start(dm_sb, diag_mask[:, :])
                    nc.gpsimd.dma_start(id_sb, ident[:, :])
                    nc.gpsimd.dma_start(lm_sb, logmult[:, :, :])
                    nc.gpsimd.dma_start(pw_sb, projw[:, :, :])

            # ========== qk projection round 0 (heads 0/1: nt 0 and 2) =====
            # ko (contraction) outer with 4 resident PSUM groups per pass;
            # two passes so the first pass's banks drain (DVE) while the
            # second computes, letting the attention pools start without
            # waiting for the full round to drain.
            with tc.tile_pool(name="p1ps", bufs=8, space="PSUM") as p1ps:
                # pass 1 (nt 0): ko outer, paced by the hT loads
                ps4 = [p1ps.tile([128, 512], F32, tag="g", name=f"q0{i}")
                       for i in range(4)]
                for ko in range(KO):
                    for i in range(4):
                        nc.tensor.matmul(
                            ps4[i],
                            w_sb[:, ko, 0:128],
                            hT[:, ko, 512 * i:512 * i + 512],
                            start=(ko == 0), stop=(ko == KO - 1),
                        )
                for i in range(4):
                    nc.vector.tensor_scalar_add(
                        qk_sb[:, 0, 512 * i:512 * i + 512], ps4[i],
                        bqk_sb[:, 0:1],
                    )
                # pass 2 (nt 2): everything resident now -> ko inner with a
                # drain right after each group, so the banks free quickly
                for i in range(4):
                    ps = p1ps.tile([128, 512], F32, tag="g", name=f"q1{i}")
                    for ko in range(KO):
                        nc.tensor.matmul(
                            ps,
                            w_sb[:, ko, 256:384],
                            hT[:, ko, 512 * i:512 * i + 512],
                            start=(ko == 0), stop=(ko == KO - 1),
                        )
                    nc.vector.tensor_scalar_add(
                        qk_sb[:, 2, 512 * i:512 * i + 512], ps,
                        bqk_sb[:, 2:3],
                    )
                # first two v tiles bridge the pool boundary: they reuse
                # drained pass-1 slots so the PE keeps running while the
                # last pass-2 groups drain (the next pool's tiles carry a
                # whole-pool WAR).
                for st in range(2):
                    ps = p1ps.tile([128, 512], F32, tag="g", name=f"vb{st}")
                    for ko in range(KO):
                        nc.tensor.matmul(
                            ps[:, 0:256],
                            hT[:, ko, 128 * st:128 * st + 128],
                            w_sb[:, ko, 512:768],
                            start=(ko == 0), stop=(ko == KO - 1),
                        )
                    nc.vector.tensor_add(
                        out=v_sb[:, st, :, 0:64],
                        in0=ps[:, 0:256].rearrange("p (h d) -> p h d", d=64),
                        in1=bv_sb.rearrange("p (h d) -> p h d", d=64),
                    )

            # ================= attention + everything else =================
            # pool order fixes which freed qk-round banks each tag aliases:
            # ppr first so the w0 v-rounds can start as soon as the first
            # qk pass drains.  av/tp are single-bank tiles whose 65/128-col
            # sub-slices rotate manually: depth-4 pipelining in one bank
            # each (PSUM allocation is bank-granular, so separate tiles
            # would blow the 8-bank budget).
            with tc.tile_pool(name="ppr", bufs=2, space="PSUM") as ppr, \
                 tc.tile_pool(name="psc", bufs=2, space="PSUM") as psc, \
                 tc.tile_pool(name="pav", bufs=1, space="PSUM") as pav, \
                 tc.tile_pool(name="ptp", bufs=1, space="PSUM") as ptp:

                av_big = pav.tile([128, 260], F32)
                tp_big = ptp.tile([128, 512], BF)
                stg_by = {}

                def emit_scores(lh, t):
                    """score pieces + exp into at bufs, then diag mask."""
                    bp = 64 * (lh % 2)
                    qn, kn = lh // 2, 2 + lh // 2
                    at_sb = at_by[lh]
                    gs = 128 * t
                    while gs < S:
                        w = min(1024, S - gs)
                        sc = psc.tile([128, 1024], F32, tag="sc")
                        o = 0
                        while o < w:
                            n = min(512, w - o)
                            nc.tensor.matmul(
                                sc[:, o:o + n],
                                qk_sb[bp:bp + 64, kn, 128 * t:128 * t + 128],
                                qk_sb[bp:bp + 64, qn, gs + o:gs + o + n],
                                start=True, stop=True,
                            )
                            o += n
                        nc.scalar.activation(
                            at_sb[:, OFF[t] + gs - 128 * t:
                                  OFF[t] + gs - 128 * t + w],
                            sc[:, :w],
                            mybir.ActivationFunctionType.Exp,
                            bias=lm_sb[:, lh, t:t + 1], scale=0.125,
                        )
                        gs += w
                    # causal 0/1 mask on the diagonal block (GPSIMD,
                    # all-SBUF, never gates ScalarE).
                    nc.gpsimd.tensor_mul(
                        out=at_by[lh][:, OFF[t]:OFF[t] + 128],
                        in0=at_by[lh][:, OFF[t]:OFF[t] + 128],
                        in1=dm_sb,
                    )

                av_rr = [0]
                tp_rr = [0]

                def emit_av(lh, t):
                    """flipped AV for q-tile t + per-partition normalize."""
                    at_sb = at_by[lh]
                    c = av_rr[0] % 4
                    av_rr[0] += 1
                    av = av_big[:, 65 * c:65 * c + 65]
                    for u in range(t + 1):
                        nc.tensor.matmul(
                            av,
                            at_sb[:, OFF[u] + 128 * (t - u):
                                  OFF[u] + 128 * (t - u) + 128],
                            v_sb[:, u, lh, :],
                            start=(u == 0), stop=(u == t),
                        )
                    rec = recp.tile([128, 1], F32, tag="rec")
                    nc.vector.reciprocal(rec, av[:, 64:65])
                    nc.vector.tensor_scalar_mul(
                        stg_by[lh // 2][:, t, lh % 2, :], av[:, 0:64], rec,
                    )

                def emit_transpose(j, st):
                    """O' [q,(e,d)] block -> ao2 [(e,d), q] via PE."""
                    c = tp_rr[0] % 4
                    tp_rr[0] += 1
                    tp = tp_big[:, 128 * c:128 * c + 128]
                    nc.tensor.matmul(
                        tp, stg_by[j][:, st, :, :], id_sb,
                        start=True, stop=True, is_transpose=True,
                    )
                    nc.vector.tensor_copy(
                        ao2[:, j, 128 * st:128 * st + 128], tp,
                    )

                def emit_cproj(st, ec):
                    pr = ppr.tile([128, 512], F32, tag="pr")
                    for jj in range(2):
                        nc.tensor.matmul(
                            pr,
                            ao2[:, jj, 128 * st:128 * st + 128],
                            pw_sb[:, jj, 512 * ec:512 * ec + 512],
                            start=(jj == 0), stop=(jj == 1),
                        )
                    o_sb = outp.tile([128, 512], BF, tag="osb")
                    if ec == 0:
                        nc.vector.tensor_copy(o_sb, pr)
                    else:
                        nc.scalar.copy(o_sb, pr)
                    nc.sync.dma_start(
                        out[128 * st:128 * st + 128,
                            512 * ec:512 * ec + 512],
                        o_sb,
                    )

                def emit_vround(st):
                    """v natural: out[s-tile, (h,d)] = hidden @ wv."""
                    ps = ppr.tile([128, 512], F32, tag="pr", name=f"v{st}")
                    for ko in range(KO):
                        nc.tensor.matmul(
                            ps[:, 0:256],
                            hT[:, ko, 128 * st:128 * st + 128],
                            w_sb[:, ko, 512:768],
                            start=(ko == 0), stop=(ko == KO - 1),
                        )
                    nc.vector.tensor_add(
                        out=v_sb[:, st, :, 0:64],
                        in0=ps[:, 0:256].rearrange("p (h d) -> p h d", d=64),
                        in1=bv_sb.rearrange("p (h d) -> p h d", d=64),
                    )

                def emit_qkr1(g):
                    """deferred qk projection for heads 2/3 (nt 1 and 3)."""
                    nt, sc4 = (1, 3)[g // 4], g % 4
                    ps = ppr.tile([128, 512], F32, tag="pr", name=f"qr{g}")
                    for ko in range(KO):
                        nc.tensor.matmul(
                            ps,
                            w_sb[:, ko, 128 * nt:128 * nt + 128],
                            hT[:, ko, 512 * sc4:512 * sc4 + 512],
                            start=(ko == 0), stop=(ko == KO - 1),
                        )
                    nc.vector.tensor_scalar_add(
                        qk_sb[:, nt, 512 * sc4:512 * sc4 + 512], ps,
                        bqk_sb[:, nt:nt + 1],
                    )

                at_by = {}

                def new_at(lh):
                    at_by[lh] = atp.tile([128, ATW], BF, tag="at",
                                         name=f"at{lh}")
                    if lh // 2 not in stg_by:
                        stg_by[lh // 2] = stgp.tile([128, NT, 2, 64], BF,
                                                    tag="stg",
                                                    name=f"stg{lh // 2}")

                # exp -> AV pipeline depth: AV for q-tile t runs LAG score
                # rounds later so transiently exp-bound rounds never stall
                # the PE on a missing exp.
                LAG = 3

                # ---- w0: head 0, v-projection rounds as PE filler ----
                new_at(0)
                for t in range(NT):
                    if t >= 2:
                        emit_vround(t)
                    emit_scores(0, t)
                    if t >= LAG:
                        emit_av(0, t - LAG)
                for u in range(NT - LAG, NT):
                    emit_av(0, u)

                # ---- w1: head 1 + all-nt1 and first-nt3 qk groups ----
                new_at(1)
                for t in range(NT):
                    if t < 5:
                        emit_qkr1(t)
                    emit_scores(1, t)
                    if t >= LAG:
                        emit_av(1, t - LAG)
                for u in range(NT - LAG, NT):
                    emit_av(1, u)

                # ---- w2: head 3; early-round PE fillers are the rest of
                # the nt3 qk groups and the pair-0 transposes ----
                new_at(3)
                for t in range(NT):
                    if 1 <= t <= 3:
                        emit_qkr1(4 + t)
                    emit_scores(3, t)
                    if 4 <= t <= 11:
                        emit_transpose(0, 2 * (t - 4))
                        emit_transpose(0, 2 * (t - 4) + 1)
                    if t >= LAG:
                        emit_av(3, t - LAG)
                for u in range(NT - LAG, NT):
                    emit_av(3, u)

                # ---- w3: head 2 + pair-1 transposes + c_proj.  ScalarE is
                # well ahead here, so AV runs at lag 2 (lag 1 late) and the
                # c_proj halves straddle the round so each PSUM drain has a
                # full round of PE work to hide behind. ----
                new_at(2)
                avdone = -1
                for t in range(NT):
                    emit_scores(2, t)
                    if t >= 4:
                        emit_cproj(t - 4, 0)
                    tgt = t - 2 if t < 10 else t - 1
                    while avdone < tgt:
                        avdone += 1
                        emit_av(2, avdone)
                    if t >= 3:
                        emit_transpose(1, t - 3)
                    if t >= 4:
                        emit_cproj(t - 4, 1)
                emit_av(2, NT - 1)
                emit_transpose(1, NT - 3)
                emit_cproj(NT - 4, 0)
                emit_transpose(1, NT - 2)
                emit_cproj(NT - 4, 1)
                emit_cproj(NT - 3, 0)
                emit_transpose(1, NT - 1)
                emit_cproj(NT - 3, 1)
                for ec in range(2):
                    emit_cproj(NT - 2, ec)
                for ec in range(2):
                    emit_cproj(NT - 1, ec)
    return nc


_NC = None


def _get_nc():
    global _NC
    if _NC is None:
        _NC = build_program()
    return _NC


# ---------------------------------------------------------------- host prep

def make_in_maps(hidden_states, c_attn_w, c_attn_b, c_proj_w):
    first_end = S // 3
    second_end = 2 * S // 3
    pos = np.arange(S)
    regions = [pos < first_end,
               (pos >= first_end) & (pos < second_end),
               pos >= second_end]
    mult = np.ones((H, S), dtype=np.float64)
    for h, r in HEAD_REGION.items():
        mult[h] = 1.0 + (FOCUS - 1.0) * regions[r].astype(np.float64)
    logm = np.log(mult).astype(np.float32)  # [H, S]

    p = np.arange(128)[:, None]
    j = np.arange(128)[None, :]
    diag = (j >= p).astype(np.float32)  # 0/1 keep-mask, applied post-exp
    iden = (j == p).astype(np.float32)

    bf = ml_dtypes.bfloat16
    in_maps = []
    for c in range(NCORES):
        b, g = divmod(c, GROUPS)
        h0 = HPC * g
        cs = slice(256 * g, 256 * g + 256)
        w_qkv = np.concatenate(
            [c_attn_w[:, cs], c_attn_w[:, 1024:2048][:, cs],
             c_attn_w[:, 2048:3072][:, cs]], axis=1,
        ).astype(bf)
        bqk_h = np.concatenate(
            [c_attn_b[cs], c_attn_b[1024:2048][cs]]
        ).reshape(4, 128).T.copy().astype(np.float32)
        bv = np.broadcast_to(
            c_attn_b[2048:3072][cs], (128, 256)
        ).astype(np.float32).copy()
        # pw2[p, j, e]: head pair j=(2j, 2j+1); p<64 -> head 2j row p,
        # p>=64 -> head 2j+1 row p-64  (matches ao2 partition interleave)
        pw = c_proj_w[64 * h0:64 * h0 + 256, :].reshape(2, 128, D)
        pw = np.ascontiguousarray(pw.transpose(1, 0, 2)).astype(bf)
        lm = logm[h0:h0 + HPC].reshape(HPC, S // 128, 128)
        lm = np.ascontiguousarray(lm.transpose(2, 0, 1)).astype(np.float32)
        in_maps.append({
            "hiddenT": np.ascontiguousarray(hidden_states[b].T).astype(bf),
            "w_qkv": w_qkv,
            "bqk": bqk_h,
            "bv_rep": bv,
            "projw": pw,
            "diag_mask": diag.astype(bf),
            "ident": iden.astype(bf),
            "logmult": lm,
        })
    return in_maps


def run_cores(in_maps, trace=False, **kw):
    from concourse.bass_utils import run_bass_kernel_spmd
    nc = _get_nc()
    return run_bass_kernel_spmd(nc, in_maps, core_ids=list(range(NCORES)),
                                trace=trace, **kw)


def kernel(hidden_states, c_attn_w, c_attn_b, c_proj_w, c_proj_b):
    hidden_states = np.asarray(hidden_states, dtype=np.float32)
    c_attn_w = np.asarray(c_attn_w, dtype=np.float32)
    c_attn_b = np.asarray(c_attn_b, dtype=np.float32)
    c_proj_w = np.asarray(c_proj_w, dtype=np.float32)
    c_proj_b = np.asarray(c_proj_b, dtype=np.float32)

    in_maps = make_in_maps(hidden_states, c_attn_w, c_attn_b, c_proj_w)
    res = run_cores(in_maps)
    out = np.zeros((B, S, D), dtype=np.float32)
    for c in range(NCORES):
        out[c // GROUPS] += np.asarray(res.results[c]["out"],
                                       dtype=np.float32)
    out += c_proj_b[None, None, :]
    return out
